# revision 1
# baseline (speedup 1.0000x reference)
"""CorefScore kernel for 8 Trainium2 NeuronCores.

Shards the mention axis M=2048 across 8 cores (256 mentions each plus a
64-row halo of preceding mentions). Per core, the banded pairwise MLP is
computed as 50 shifted elementwise products X^T * shift(X^T, delta) (DVE,
fp16, batched 8 deltas per op) contracted with W1c on the PE in fp16 with
fp32 PSUM accumulation; the Ya + shift(Yb) term is merged on DVE and added
into PSUM via an identity matmul. ReLU activations are stored in large SBUF
buffers; the w2p contraction runs as a deferred back-to-back matmul phase so
the PE stream never waits on ScalarE mid-round. Masking/dummy column are
applied with host-precomputed mask tensors.
"""

import os
import sys

import numpy as np

for _p in ("/opt/trn_rl_repo", "/opt/pypackages"):
    if os.path.isdir(_p) and _p not in sys.path:
        sys.path.append(_p)

import concourse.bacc as bacc
import concourse.bass as bass
import concourse.mybir as mybir
import concourse.tile as tile
from concourse.ap import AP
from concourse.bass_utils import run_bass_kernel_spmd

F16 = mybir.dt.float16
F32 = mybir.dt.float32
AF = mybir.ActivationFunctionType

M, D, H, K = 2048, 900, 150, 50
NCORES = 8
MC = M // NCORES          # owned mentions per core
HB = 64                   # halo columns (>= K)
W = MC + HB               # X^T window width per core
DP = 1024                 # padded feature dim (8 tiles of 128)
NDT = DP // 128           # number of d tiles
G = 2                     # deltas per PSUM round
NR = K // G               # rounds
GP = 8                    # deltas per product batch
H1, H2 = 128, H - 128     # h split
KM = K * MC

_cache = {}


def _ap3(t_ap, p_lo, p_n, off, dims):
    """3-D free-dim view of a tile AP: partitions [p_lo, p_lo+p_n), free
    offset `off` elements, free dims = [(stride, n), ...]."""
    b = t_ap[p_lo:p_lo + p_n, 0:1]
    pstride = b.ap[0][0]
    return AP(b.tensor, b.offset + off, [[pstride, p_n]] + [list(d) for d in dims])


def _build():
    nc = bacc.Bacc("TRN2", target_bir_lowering=False, debug=False)

    xt_d = nc.dram_tensor("xt", [DP, W], F16, kind="ExternalInput").ap()
    w1c_d = nc.dram_tensor("w1c", [DP, H], F16, kind="ExternalInput").ap()
    w1a_d = nc.dram_tensor("w1a", [DP, H], F16, kind="ExternalInput").ap()
    w1b_d = nc.dram_tensor("w1b", [DP, H], F16, kind="ExternalInput").ap()
    w1m_d = nc.dram_tensor("w1m", [DP, H], F16, kind="ExternalInput").ap()
    w2m_d = nc.dram_tensor("w2m", [H, 1], F16, kind="ExternalInput").ap()
    w2p1_d = nc.dram_tensor("w2p1", [H1, 1], F16, kind="ExternalInput").ap()
    w2p2_d = nc.dram_tensor("w2p2", [33, 1], F16, kind="ExternalInput").ap()
    idn_d = nc.dram_tensor("idn", [128, 128], F16, kind="ExternalInput").ap()
    b1m_d = nc.dram_tensor("b1mc", [H, 1], F32, kind="ExternalInput").ap()
    b1p_d = nc.dram_tensor("b1pc", [H, 1], F32, kind="ExternalInput").ap()
    mmul_d = nc.dram_tensor("mmul", [MC, K + 1], F32, kind="ExternalInput").ap()
    madd_d = nc.dram_tensor("madd", [MC, K + 1], F32, kind="ExternalInput").ap()
    out_d = nc.dram_tensor("out", [MC, K + 1], F32, kind="ExternalOutput").ap()

    hsl = [(0, H1), (H1, H2)]  # (h offset, h size) per h tile

    with tile.TileContext(nc) as tc:
        with (
            tc.tile_pool(name="const", bufs=1) as cp,
            tc.tile_pool(name="work", bufs=4) as wp,
            tc.tile_pool(name="ps_pre", bufs=2, space="PSUM") as pp_pre,
            tc.tile_pool(name="ps_a1", bufs=2, space="PSUM") as pp_a1,
            tc.tile_pool(name="ps_a2", bufs=2, space="PSUM") as pp_a2,
            tc.tile_pool(name="ps_pair", bufs=2, space="PSUM") as pp_pair,
        ):
            # ---- load inputs ----
            xts = []
            for t in range(NDT):
                xt = cp.tile([128, W], F16, tag=f"xt{t}")
                nc.sync.dma_start(out=xt[:], in_=xt_d[128 * t:128 * (t + 1), :])
                xts.append(xt)

            def load_w(dram, name):
                ts = []
                for t in range(NDT):
                    w = cp.tile([128, H], F16, tag=f"{name}{t}")
                    nc.sync.dma_start(out=w[:], in_=dram[128 * t:128 * (t + 1), :])
                    ts.append(w)
                return ts

            w1c_s = load_w(w1c_d, "w1c")
            w1a_s = load_w(w1a_d, "w1a")
            w1b_s = load_w(w1b_d, "w1b")
            w1m_s = load_w(w1m_d, "w1m")

            w2m1 = cp.tile([H1, 1], F16, tag="w2m1")
            nc.sync.dma_start(out=w2m1[:], in_=w2m_d[0:H1, :])
            w2m2 = cp.tile([H2, 1], F16, tag="w2m2")
            nc.sync.dma_start(out=w2m2[:], in_=w2m_d[H1:H, :])
            w2p1 = cp.tile([H1, 1], F16, tag="w2p1")
            nc.sync.dma_start(out=w2p1[:], in_=w2p1_d[:])
            w2p2 = cp.tile([33, 1], F16, tag="w2p2")
            nc.sync.dma_start(out=w2p2[:], in_=w2p2_d[:])
            idn = cp.tile([128, 128], F16, tag="idn")
            nc.sync.dma_start(out=idn[:], in_=idn_d[:])
            b1m_c = []
            b1p_c = []
            for h, (ho, hn) in enumerate(hsl):
                bm = cp.tile([hn, 1], F32, tag=f"b1m{h}")
                nc.sync.dma_start(out=bm[:], in_=b1m_d[ho:ho + hn, :])
                b1m_c.append(bm)
                bp = cp.tile([hn, 1], F32, tag=f"b1p{h}")
                nc.sync.dma_start(out=bp[:], in_=b1p_d[ho:ho + hn, :])
                b1p_c.append(bp)
            mm_sb = []
            ma_sb = []
            for mb in range(2):
                mm = cp.tile([128, K + 1], F32, tag=f"mm{mb}")
                nc.sync.dma_start(out=mm[:], in_=mmul_d[128 * mb:128 * (mb + 1), :])
                mm_sb.append(mm)
                ma = cp.tile([128, K + 1], F32, tag=f"ma{mb}")
                nc.sync.dma_start(out=ma[:], in_=madd_d[128 * mb:128 * (mb + 1), :])
                ma_sb.append(ma)

            # product helper: P[d, j, m] = X^T[d, m] * X^T[d, m - (d0+j)]
            def emit_products(d0):
                pts = []
                for t in range(NDT):
                    pt = wp.tile([128, G * MC], F16, tag=f"p{t}")
                    nc.vector.tensor_tensor(
                        _ap3(pt[:], 0, 128, 0, [(MC, G), (1, MC)]),
                        _ap3(xts[t][:], 0, 128, HB, [(0, G), (1, MC)]),
                        _ap3(xts[t][:], 0, 128, HB - d0, [(-1, G), (1, MC)]),
                        mybir.AluOpType.mult)
                    pts.append(pt)
                return pts

            # prologue: products for the first rounds, so DVE runs ahead of
            # the PE while the PE is busy with the mention-score MLP below
            PRO = 2
            pts_queue = [emit_products(K - 1 - G * r) for r in range(PRO)]

            # ---- Ya (owned window, + b1p) and Yb (full window) ----
            ya = []
            yb = []
            for h, (ho, hn) in enumerate(hsl):
                psya = pp_pre.tile([hn, MC], F32, tag="pre_ps")
                for t in range(NDT):
                    nc.tensor.matmul(psya[:], w1a_s[t][:, ho:ho + hn], xts[t][:, HB:W],
                                     start=(t == 0), stop=(t == NDT - 1))
                y = cp.tile([hn, MC], F16, tag=f"ya{h}")
                nc.scalar.activation(y[:], psya[:], AF.Identity, bias=b1p_c[h][:])
                ya.append(y)
                psyb = pp_pre.tile([hn, W], F32, tag="pre_ps")
                for t in range(NDT):
                    nc.tensor.matmul(psyb[:], w1b_s[t][:, ho:ho + hn], xts[t][:],
                                     start=(t == 0), stop=(t == NDT - 1))
                y = cp.tile([hn, W], F16, tag=f"yb{h}")
                nc.scalar.copy(y[:], psyb[:])
                yb.append(y)

            # ---- mention score MLP over the full window ----
            ment_act = []
            for h, (ho, hn) in enumerate(hsl):
                psm = pp_pre.tile([hn, W], F32, tag="pre_ps")
                for t in range(NDT):
                    nc.tensor.matmul(psm[:], w1m_s[t][:, ho:ho + hn], xts[t][:],
                                     start=(t == 0), stop=(t == NDT - 1))
                ma = cp.tile([hn, W], F16, tag=f"mact{h}")
                nc.scalar.activation(ma[:], psm[:], AF.Relu, bias=b1m_c[h][:])
                ment_act.append(ma)
            psme = pp_pre.tile([1, W], F32, tag="pre_ps")
            nc.tensor.matmul(psme[:], w2m1[:], ment_act[0][:], start=True, stop=False)
            nc.tensor.matmul(psme[:], w2m2[:], ment_act[1][:], start=False, stop=True)
            ment_row = cp.tile([1, W], F16, tag="mentrow")
            nc.scalar.copy(ment_row[:], psme[:])

            # merge helper: C[h, j, m] = Ya'[h, m] + Yb[h, m - (d0+j)]
            def emit_merges(d0):
                c1 = wp.tile([H1, G * MC], F16, tag="c1")
                nc.vector.tensor_tensor(
                    _ap3(c1[:], 0, H1, 0, [(MC, G), (1, MC)]),
                    _ap3(ya[0][:], 0, H1, 0, [(0, G), (1, MC)]),
                    _ap3(yb[0][:], 0, H1, HB - d0, [(-1, G), (1, MC)]),
                    mybir.AluOpType.add)
                c2 = wp.tile([H2, G * MC], F16, tag="c2")
                nc.vector.tensor_tensor(
                    _ap3(c2[:], 0, H2, 0, [(MC, G), (1, MC)]),
                    _ap3(ya[1][:], 0, H2, 0, [(0, G), (1, MC)]),
                    _ap3(yb[1][:], 0, H2, HB - d0, [(-1, G), (1, MC)]),
                    mybir.AluOpType.add)
                return c1, c2

            cms_queue = [emit_merges(K - 1 - G * r) for r in range(PRO)]

            # ---- banded pairwise loop: G deltas per round ----
            GW = G * MC
            KH = 32  # k rows completed after the first KH//G rounds (32-aligned)
            pair_flat = cp.tile([1, KM], F16, tag="pairflat")
            pairK = cp.tile([K, MC], F16, tag="pairK")
            pstr0 = pp_pre.tile([128, K], F16, tag="pre_ps")
            pstr1 = pp_pre.tile([128, K], F16, tag="pre_ps")
            pstr = [pstr0, pstr1]
            a2x_bufs = []
            for i in range(2):
                ab = cp.tile([33, GW], F16, tag=f"a2x{i}")
                nc.vector.memset(ab[:], 0.0)
                a2x_bufs.append(ab)
            for r in range(NR):
                d0 = K - 1 - G * r  # deltas d0 .. d0+G-1 (descending rounds)

                if r + PRO < NR:
                    pts_queue.append(emit_products(K - 1 - G * (r + PRO)))
                    cms_queue.append(emit_merges(K - 1 - G * (r + PRO)))
                pts = pts_queue.pop(0)
                c1, c2 = cms_queue.pop(0)

                # A = P @ W1c + C, per h tile, PSUM-accumulated
                ps1 = pp_a1.tile([H1, GW], F32, tag="a1")
                for t in range(NDT):
                    nc.tensor.matmul(ps1[:], w1c_s[t][:, 0:H1], pts[t][:],
                                     start=(t == 0), stop=False)
                nc.tensor.matmul(ps1[:], idn[0:H1, 0:H1], c1[:], start=False, stop=True)
                ps2 = pp_a2.tile([H2, GW], F32, tag="a2")
                for t in range(NDT):
                    nc.tensor.matmul(ps2[:], w1c_s[t][:, H1:H], pts[t][:],
                                     start=(t == 0), stop=False)
                nc.tensor.matmul(ps2[:], idn[0:H2, 0:H2], c2[:], start=False, stop=True)

                # relu evacuation (b1p already inside C via Ya)
                a1 = wp.tile([H1, GW], F16, tag="a1sb")
                nc.scalar.activation(a1[:], ps1[:], AF.Relu)
                a2x = a2x_bufs[r % 2]
                nc.scalar.activation(a2x[0:H2, :], ps2[:], AF.Relu)
                # ment_j carrier row (at partition 32; w2p2 rows 22..31 are 0)
                nc.scalar.copy(
                    _ap3(a2x[:], 32, 1, 0, [(MC, G), (1, MC)]),
                    _ap3(ment_row[:], 0, 1, HB - d0, [(-1, G), (1, MC)]))

                # pair = w2p . A  (+ ment_j via carrier)
                psp = pp_pair.tile([1, GW], F32, tag="pair")
                nc.tensor.matmul(psp[:], w2p1[:], a1[:], start=True, stop=False)
                nc.tensor.matmul(psp[:], w2p2[:], a2x[:], start=False, stop=True)
                for j in range(G):
                    k = K - (d0 + j)
                    nc.scalar.copy(pair_flat[0:1, MC * k:MC * (k + 1)],
                                   psp[0:1, MC * j:MC * (j + 1)])
                if r == KH // G - 1:
                    nc.sync.dma_start(
                        out=pairK[0:KH, :],
                        in_=_ap3(pair_flat[:], 0, 1, 0, [(MC, KH), (1, MC)]))
                    for mb in range(2):
                        nc.tensor.transpose(pstr[mb][:, 0:KH],
                                            pairK[0:KH, 128 * mb:128 * (mb + 1)],
                                            idn[0:KH, 0:KH])

            # ment as per-partition columns for the owned 2x128 mention blocks
            ment_col = []
            for mb in range(2):
                pst = pp_pair.tile([128, 1], F16, tag="pair")
                nc.tensor.transpose(pst[:], ment_row[0:1, HB + 128 * mb:HB + 128 * (mb + 1)],
                                    idn[0:1, 0:1])
                mc = cp.tile([128, 1], F32, tag=f"mcol{mb}")
                nc.scalar.copy(mc[:], pst[:])
                ment_col.append(mc)

            # ---- respread (k-major) to rows, transpose, mask, store ----
            # first KH k-rows were finished mid-loop (descending deltas), so
            # their respread + transpose overlap the remaining band rounds
            kw = pp_pair.tile([1, GW], F32, tag="pair")
            nc.tensor.matmul(kw[0:1, 0:128], idn[:, 0:1], idn[:, :],
                             start=True, stop=True)
            nc.sync.dma_start(
                out=pairK[KH:K, :],
                in_=_ap3(pair_flat[:], 0, 1, KH * MC, [(MC, K - KH), (1, MC)]))
            kw2 = pp_pair.tile([1, GW], F32, tag="pair")
            nc.tensor.matmul(kw2[0:1, 0:128], idn[KH:K, 0:1],
                             pairK[KH:K, 0:128], start=True, stop=True)
            for mb in range(2):
                nc.tensor.transpose(pstr[mb][:, KH:K], pairK[KH:K, 128 * mb:128 * (mb + 1)],
                                    idn[KH:K, KH:K])
                sc = wp.tile([128, K + 1], F32, tag=f"sc{mb}")
                nc.vector.memset(sc[:], 0.0)
                nc.scalar.activation(sc[:, 0:K], pstr[mb][:], AF.Identity,
                                     bias=ment_col[mb][:])
                nc.vector.tensor_mul(sc[:], sc[:], mm_sb[mb][:])
                nc.vector.tensor_add(sc[:], sc[:], ma_sb[mb][:])
                nc.sync.dma_start(out=out_d[128 * mb:128 * (mb + 1), :], in_=sc[:])

    nc.compile()
    return nc


def _prep_inputs(inputs):
    X = np.ascontiguousarray(inputs["mention_reprs"], dtype=np.float32)
    assert X.shape == (M, D)
    w1p = np.asarray(inputs["w1p"], dtype=np.float32)
    W1a, W1b, W1c = w1p[:D], w1p[D:2 * D], w1p[2 * D:]
    f16 = lambda a: np.ascontiguousarray(a, dtype=np.float16)

    def padD(w):  # [D, H] -> [DP, H] fp16
        out = np.zeros((DP, H), dtype=np.float16)
        out[:D] = w.astype(np.float16)
        return out

    xtp = np.zeros((DP, M + HB), dtype=np.float16)
    xtp[:D, HB:] = X.T.astype(np.float16)

    w2p = np.asarray(inputs["w2p"], dtype=np.float32)
    shared = {
        "w1c": padD(W1c),
        "w1a": padD(W1a),
        "w1b": padD(W1b),
        "w1m": padD(np.asarray(inputs["w1m"], dtype=np.float32)),
        "w2m": f16(np.asarray(inputs["w2m"], dtype=np.float32).reshape(H, 1)),
        "w2p1": f16(w2p[:H1].reshape(H1, 1)),
        "w2p2": f16(np.concatenate([w2p[H1:], np.zeros(10, np.float32),
                                    [1.0]]).reshape(33, 1)),
        "idn": np.eye(128, dtype=np.float16),
        "b1mc": np.ascontiguousarray(
            np.asarray(inputs["b1m"], dtype=np.float32).reshape(H, 1)),
        "b1pc": np.ascontiguousarray(
            np.asarray(inputs["b1p"], dtype=np.float32).reshape(H, 1)),
    }

    b2m = float(np.asarray(inputs["b2m"]).reshape(-1)[0])
    b2p = float(np.asarray(inputs["b2p"]).reshape(-1)[0])
    in_maps = []
    for c in range(NCORES):
        r0 = MC * c
        xt_c = np.ascontiguousarray(xtp[:, r0:r0 + W])
        mmul = np.ones((MC, K + 1), dtype=np.float32)
        madd = np.full((MC, K + 1), np.float32(b2p + 2.0 * b2m), dtype=np.float32)
        mmul[:, K] = 0.0
        madd[:, K] = 0.0
        if c == 0:
            for i in range(min(K, MC)):
                mmul[i, :K - i] = 0.0
                madd[i, :K - i] = np.float32(-1e9)
        in_maps.append({"xt": xt_c, "mmul": mmul, "madd": madd, **shared})
    return in_maps


def _get_nc(inputs):
    if "nc" not in _cache:
        _cache["nc"] = _build()
    return _cache["nc"]


def _run(inputs, trace=False):
    assert int(np.asarray(inputs["K"])) == K
    nc = _get_nc(inputs)
    in_maps = _prep_inputs(inputs)
    res = run_bass_kernel_spmd(nc, in_maps, list(range(NCORES)), trace=trace)
    out = np.concatenate([res.results[c]["out"] for c in range(NCORES)], axis=0)
    return out.astype(np.float32), res


def kernel(**inputs) -> np.ndarray:
    out, _ = _run(inputs, trace=False)
    return out



# revision 12
# speedup vs baseline: 1.4211x; 1.4211x over previous
"""CorefScore kernel for 8 Trainium2 NeuronCores.

Shards the mention axis M=2048 across 8 cores (256 owned mentions plus a
64-row halo of preceding mentions). The banded pairwise MLP runs in
"superrounds" of up to 4 rounds (2 deltas each): the DVE emits one batched
shifted-product tensor_tensor per d-tile per superround ([128, 8*256] fp16),
the PE contracts them with W1c in fp16 (fp32 PSUM); the Ya + shift(Yb) terms
are injected straight into PSUM via identity matmuls with strided moving
operands (no DVE merge for the 128-wide h half). The 22-wide h2 half of all
4 rounds is packed into one PSUM bank at partition offsets 0/32/64/96 via
col-tiled matmuls that execute concurrently on disjoint PE column strips.
Pair scores (w2p contraction) use diagonal 32x32 tiles, also packed 4 rounds
per PSUM bank, and are respread/transposed per superround so only a tiny
tail remains after the last round. Inputs load via a handful of large DMAs
split across the Sync and ScalarE HWDGE queues.
"""

import os
import sys

import numpy as np

for _p in ("/opt/trn_rl_repo", "/opt/pypackages"):
    if os.path.isdir(_p) and _p not in sys.path:
        sys.path.append(_p)

import concourse.bacc as bacc
import concourse.bass as bass
import concourse.mybir as mybir
import concourse.tile as tile
from concourse.ap import AP
from concourse.bass_utils import run_bass_kernel_spmd

F16 = mybir.dt.float16
F32 = mybir.dt.float32
AF = mybir.ActivationFunctionType

M, D, H, K = 2048, 900, 150, 50
NCORES = 8
MC = M // NCORES          # owned mentions per core
HB = 64                   # halo columns (>= K)
W = MC + HB               # X^T window width per core
DP = 1024                 # padded feature dim (8 tiles of 128)
NDT = DP // 128           # number of d tiles
H1, H2 = 128, H - 128     # h split
# superround sizes (rounds of 2 deltas each); sum = 25 rounds = 50 deltas
SRS = [2, 4, 4, 4, 4, 4, 3]

_cache = {}


def _ap3(t_ap, p_lo, p_n, off, dims, pstep=1):
    """3-D free-dim view of a tile AP: partitions [p_lo, p_lo+p_n*pstep) with
    partition step pstep, free offset `off` elements, free dims."""
    b = t_ap[p_lo:p_lo + 1, 0:1]
    pstride = b.ap[0][0]
    return AP(b.tensor, b.offset + off,
              [[pstride * pstep, p_n]] + [list(d) for d in dims])


def _build():
    nc = bacc.Bacc("TRN2", target_bir_lowering=False, debug=False)

    xt_d = nc.dram_tensor("xt", [DP, W], F16, kind="ExternalInput").ap()
    w1a_d = nc.dram_tensor("w1a", [DP, H], F16, kind="ExternalInput").ap()
    w1b_d = nc.dram_tensor("w1b", [DP, H], F16, kind="ExternalInput").ap()
    w1c_d = nc.dram_tensor("w1c", [DP, H], F16, kind="ExternalInput").ap()
    w1m_d = nc.dram_tensor("w1m", [DP, H], F16, kind="ExternalInput").ap()
    bias_d = nc.dram_tensor("bias_all", [128, 4], F32, kind="ExternalInput").ap()
    w2_d = nc.dram_tensor("w2_all", [128, 4], F16, kind="ExternalInput").ap()
    idn_d = nc.dram_tensor("idn", [128, 128], F16, kind="ExternalInput").ap()
    mm_d = nc.dram_tensor("mm_ma", [128, 4 * (K + 1)], F32,
                          kind="ExternalInput").ap()
    out_d = nc.dram_tensor("out", [MC, K + 1], F32, kind="ExternalOutput").ap()

    def dma3(eng, dst_tile, src_dram, cols):
        """One DMA loading [DP, cols] dram into a [128, NDT*cols] tile."""
        src = AP(src_dram.tensor, src_dram.offset,
                 [[cols, 128], [128 * cols, NDT], [1, cols]])
        eng.dma_start(out=dst_tile[:, 0:NDT * cols], in_=src)

    with tile.TileContext(nc) as tc:
        with (
            tc.tile_pool(name="cp", bufs=1) as cp,
            tc.tile_pool(name="wp", bufs=2) as wp,
            tc.tile_pool(name="pp", bufs=2, space="PSUM") as pp,
        ):
            # ---- input loads: sync queue ----
            xt = cp.tile([128, NDT * W], F16, tag="xt")
            dma3(nc.sync, xt, xt_d, W)
            bias = cp.tile([128, 4], F32, tag="bias")
            nc.sync.dma_start(out=bias[:], in_=bias_d[:])
            idn = cp.tile([128, 128], F16, tag="idn")
            nc.sync.dma_start(out=idn[:], in_=idn_d[:])
            w2 = cp.tile([128, 4], F16, tag="w2")
            nc.sync.dma_start(out=w2[:], in_=w2_d[:])
            mm_ma = cp.tile([128, 4 * (K + 1)], F32, tag="mm_ma")
            nc.sync.dma_start(out=mm_ma[:], in_=mm_d[:])
            # ---- input loads: scalar (Act) HWDGE queue ----
            w1a = cp.tile([128, NDT * H], F16, tag="w1a")
            dma3(nc.scalar, w1a, w1a_d, H)
            w1b = cp.tile([128, NDT * H], F16, tag="w1b")
            dma3(nc.scalar, w1b, w1b_d, H)
            w1m = cp.tile([128, NDT * H], F16, tag="w1m")
            dma3(nc.scalar, w1m, w1m_d, H)
            w1c = cp.tile([128, NDT * H], F16, tag="w1c")
            dma3(nc.scalar, w1c, w1c_d, H)

            def wsl(wt, t, ho, hn):  # stationary slice of a weight tile
                return wt[:, H * t + ho:H * t + ho + hn]

            # ---- preamble MLPs: Ya (owned, +b1p), Yb (window), mention ----
            psya1 = pp.tile([H1, MC], F32, tag="ah1")
            psya2 = pp.tile([H2, MC], F32, tag="ah1")
            psyb1 = pp.tile([H1, W], F32, tag="ah2")
            psyb2 = pp.tile([H2, W], F32, tag="ah2")
            psm1 = pp.tile([H1, W], F32, tag="psp")
            psm2 = pp.tile([H2, W], F32, tag="psp")
            for t in range(NDT):
                nc.tensor.matmul(psya1[:], wsl(w1a, t, 0, H1),
                                 xt[:, W * t + HB:W * (t + 1)],
                                 start=(t == 0), stop=(t == NDT - 1))
            for t in range(NDT):
                nc.tensor.matmul(psya2[:], wsl(w1a, t, H1, H2),
                                 xt[:, W * t + HB:W * (t + 1)],
                                 start=(t == 0), stop=(t == NDT - 1))
            for t in range(NDT):
                nc.tensor.matmul(psyb1[:], wsl(w1b, t, 0, H1),
                                 xt[:, W * t:W * (t + 1)],
                                 start=(t == 0), stop=(t == NDT - 1))
            for t in range(NDT):
                nc.tensor.matmul(psyb2[:], wsl(w1b, t, H1, H2),
                                 xt[:, W * t:W * (t + 1)],
                                 start=(t == 0), stop=(t == NDT - 1))
            for t in range(NDT):
                nc.tensor.matmul(psm1[:], wsl(w1m, t, 0, H1),
                                 xt[:, W * t:W * (t + 1)],
                                 start=(t == 0), stop=(t == NDT - 1))
            for t in range(NDT):
                nc.tensor.matmul(psm2[:], wsl(w1m, t, H1, H2),
                                 xt[:, W * t:W * (t + 1)],
                                 start=(t == 0), stop=(t == NDT - 1))

            ya1 = cp.tile([H1, MC], F16, tag="ya1")
            nc.scalar.activation(ya1[:], psya1[:], AF.Identity, bias=bias[:, 2:3])
            ya2 = cp.tile([H2, MC], F16, tag="ya2")
            nc.scalar.activation(ya2[:], psya2[:], AF.Identity,
                                 bias=bias[0:H2, 3:4])
            yb1 = cp.tile([H1, W], F16, tag="yb1")
            nc.scalar.copy(yb1[:], psyb1[:])
            yb2 = cp.tile([H2, W], F16, tag="yb2")
            nc.scalar.copy(yb2[:], psyb2[:])
            ma1 = cp.tile([H1, W], F16, tag="ma1")
            nc.scalar.activation(ma1[:], psm1[:], AF.Relu, bias=bias[:, 0:1])
            ma2 = cp.tile([H2, W], F16, tag="ma2")
            nc.scalar.activation(ma2[:], psm2[:], AF.Relu, bias=bias[0:H2, 1:2])

            # mention score row over the window
            psme = pp.tile([1, W], F32, tag="pre")
            nc.tensor.matmul(psme[:], w2[:, 0:1], ma1[:], start=True, stop=False)
            nc.tensor.matmul(psme[:], w2[0:H2, 1:2], ma2[:], start=False,
                             stop=True)
            # row 0 = mention scores; rows 1..31 stay zero so the e0-column
            # identity-inject matmul (32-wide contraction) picks row 0 only
            ment_row = cp.tile([32, W], F16, tag="ment_row")
            nc.vector.memset(ment_row[:], 0.0)
            nc.scalar.copy(ment_row[0:1, :], psme[:])
            # ment as per-partition columns for the owned 2x128 mention blocks
            ment_col = []
            for mb in range(2):
                pst = pp.tile([128, 1], F16, tag="pre")
                nc.tensor.transpose(pst[:],
                                    ment_row[0:1, HB + 128 * mb:HB + 128 * (mb + 1)],
                                    idn[0:1, 0:1])
                mcol = cp.tile([128, 1], F32, tag=f"mcol{mb}")
                nc.scalar.copy(mcol[:], pst[:])
                ment_col.append(mcol)

            scm = []
            for mb in range(2):
                s = cp.tile([128, K + 1], F32, tag=f"scm{mb}")
                nc.vector.memset(s[:], 0.0)
                scm.append(s)

            # ---- DVE: batched shifted products per superround ----
            # SR s covers rounds r0..r0+R-1; delta of column group j (0..2R-1)
            # is d0 - j with d0 = K - 2*r0; product col j*MC+m multiplies
            # X^T[., m] (owned) by X^T[., m - (d0 - j)].
            r0s = []
            acc = 0
            for R in SRS:
                r0s.append(acc)
                acc += R

            def emit_products(s):
                R = SRS[s]
                d0 = K - 2 * r0s[s]
                pts = []
                for t in range(NDT):
                    pt = wp.tile([128, 2 * R * MC], F16, tag=f"pt{t}",
                                 padded_shape=[128, 8 * MC])
                    nc.vector.tensor_tensor(
                        _ap3(pt[:], 0, 128, 0, [(MC, 2 * R), (1, MC)]),
                        _ap3(xt[:], 0, 128, W * t + HB, [(0, 2 * R), (1, MC)]),
                        _ap3(xt[:], 0, 128, W * t + HB - d0, [(1, 2 * R), (1, MC)]),
                        mybir.AluOpType.mult)
                    pts.append(pt)
                return pts

            def emit_c2(s):
                R = SRS[s]
                d0 = K - 2 * r0s[s]
                c2 = wp.tile([H2, 2 * R * MC], F16, tag="c2",
                             padded_shape=[128, 8 * MC])
                nc.vector.tensor_tensor(
                    _ap3(c2[:], 0, H2, 0, [(MC, 2 * R), (1, MC)]),
                    _ap3(ya2[:], 0, H2, 0, [(0, 2 * R), (1, MC)]),
                    _ap3(yb2[:], 0, H2, HB - d0, [(1, 2 * R), (1, MC)]),
                    mybir.AluOpType.add)
                return c2

            pts_q = {0: emit_products(0)}
            c2_q = {0: emit_c2(0)}
            pts_q[1] = emit_products(1)

            # ---- superround loop state ----
            a1_ring = []      # (sr, g) -> a1 tile, kept one SR back
            prev = None       # state of SR s-1 for deferred pair work

            pairK_pool = wp   # per-SR respread tiles

            def emit_pairs_for(state):
                """PE pair matmuls + evac + respread + transposes for SR s-1."""
                s, R, a1s, a2xs = state
                d0 = K - 2 * r0s[s]
                psp = pp.tile([97, MC * 2], F32, tag="psp")
                for g in range(R):
                    # ment_j first (start=True): a strided-moving matmul with
                    # start=False wedges the exec unit, so it opens the group
                    nc.tensor.matmul(
                        psp[32 * g:32 * g + 1, :], idn[0:32, 0:1],
                        _ap3(ment_row[:], 0, 32, HB - (d0 - 2 * g),
                             [(1, 2), (1, MC)]),
                        start=True, stop=False, tile_position=(0, 32 * g))
                    nc.tensor.matmul(psp[32 * g:32 * g + 1, :], w2[:, 2:3],
                                     a1s[g][:], start=False, stop=False,
                                     tile_position=(0, 32 * g))
                    nc.tensor.matmul(psp[32 * g:32 * g + 1, :],
                                     w2[32 * g:32 * g + H2, 3:4],
                                     a2xs[32 * g:32 * g + H2, :],
                                     start=False, stop=True,
                                     tile_position=(32 * g, 32 * g))
                pair_sb = wp.tile([97, MC * 2], F16, tag="pair_sb")
                nc.scalar.copy(pair_sb[:], psp[:])
                pairK = pairK_pool.tile([2 * R, MC], F16, tag="pairK",
                                        padded_shape=[128, MC])
                nc.sync.dma_start(
                    out=pairK[:],
                    in_=_ap3(pair_sb[:], 0, R, 0, [(MC, 2), (1, MC)], pstep=32))
                return pairK

            def emit_respread_tail(state, pairK):
                """Transposes + scM chunk evac for SR s-1 (after its DMA)."""
                s, R, a1s, a2xs = state
                k0 = 2 * r0s[s]
                cn = 2 * R
                for mb in range(2):
                    ptr = pp.tile([128, cn], F16, tag="pre",
                                  padded_shape=[128, 8])
                    nc.tensor.transpose(ptr[:], pairK[0:cn, 128 * mb:128 * (mb + 1)],
                                        idn[0:cn, 0:cn])
                    nc.scalar.activation(scm[mb][:, k0:k0 + cn], ptr[:],
                                         AF.Identity, bias=ment_col[mb][:])

            for s, R in enumerate(SRS):
                r0 = r0s[s]
                d0 = K - 2 * r0
                pts = pts_q.pop(s)
                c2 = c2_q.pop(s)

                # deferred pair matmuls for the previous superround
                pairK_prev = emit_pairs_for(prev) if prev is not None else None

                # per-round h1 streams (+ Ya / shifted-Yb injections)
                a1s = []
                for g in range(R):
                    ps1 = pp.tile([H1, 2 * MC], F32, tag="ah1")
                    for t in range(NDT):
                        nc.tensor.matmul(ps1[:], wsl(w1c, t, 0, H1),
                                         pts[t][:, 2 * g * MC:(2 * g + 2) * MC],
                                         start=(t == 0), stop=False)
                    nc.tensor.matmul(
                        ps1[:], idn[:],
                        _ap3(ya1[:], 0, 128, 0, [(0, 2), (1, MC)]),
                        start=False, stop=False)
                    nc.tensor.matmul(
                        ps1[:], idn[:],
                        _ap3(yb1[:], 0, 128, HB - (d0 - 2 * g), [(1, 2), (1, MC)]),
                        start=False, stop=True)
                    a1 = wp.tile([H1, 2 * MC], F16, tag="a1", bufs=8)
                    nc.scalar.activation(a1[:], ps1[:], AF.Relu)
                    a1s.append(a1)

                # packed h2: col-tiled matmuls, 4 rounds -> one PSUM bank
                hp = 32 * (R - 1) + H2
                ps2 = pp.tile([hp, 2 * MC], F32, tag="ah2")
                for t in range(NDT):
                    for g in range(R):
                        nc.tensor.matmul(ps2[32 * g:32 * g + H2, :],
                                         wsl(w1c, t, H1, H2),
                                         pts[t][:, 2 * g * MC:(2 * g + 2) * MC],
                                         start=(t == 0), stop=False,
                                         tile_position=(0, 32 * g))
                for g in range(R):
                    nc.tensor.matmul(ps2[32 * g:32 * g + H2, :],
                                     idn[0:H2, 0:H2],
                                     c2[0:H2, 2 * g * MC:(2 * g + 2) * MC],
                                     start=False, stop=True,
                                     tile_position=(0, 32 * g))
                a2x = wp.tile([hp, 2 * MC], F16, tag="a2x",
                              padded_shape=[128, 2 * MC])
                nc.scalar.activation(a2x[:], ps2[:], AF.Relu)

                # respread tail for SR s-1 (its DMA has landed by now)
                if prev is not None:
                    emit_respread_tail(prev, pairK_prev)

                # prefetch DVE work for s+2 / c2 for s+1
                if s + 1 < len(SRS):
                    c2_q[s + 1] = emit_c2(s + 1)
                if s + 2 < len(SRS):
                    pts_q[s + 2] = emit_products(s + 2)

                prev = (s, R, a1s, a2x[:])

            # ---- epilogue: pairs of the last superround + final masking ----
            pairK_last = emit_pairs_for(prev)
            emit_respread_tail(prev, pairK_last)
            for mb in range(2):
                nc.vector.tensor_mul(scm[mb][:], scm[mb][:],
                                     mm_ma[:, (K + 1) * mb:(K + 1) * (mb + 1)])
                nc.vector.tensor_add(
                    scm[mb][:], scm[mb][:],
                    mm_ma[:, (K + 1) * (2 + mb):(K + 1) * (3 + mb)])
                nc.sync.dma_start(out=out_d[128 * mb:128 * (mb + 1), :],
                                  in_=scm[mb][:])

    nc.compile()
    return nc


def _prep_inputs(inputs):
    X = np.ascontiguousarray(inputs["mention_reprs"], dtype=np.float32)
    assert X.shape == (M, D)
    w1p = np.asarray(inputs["w1p"], dtype=np.float32)
    W1a, W1b, W1c = w1p[:D], w1p[D:2 * D], w1p[2 * D:]

    def padD(w):  # [D, H] -> [DP, H] fp16
        out = np.zeros((DP, H), dtype=np.float16)
        out[:D] = w.astype(np.float16)
        return out

    xtp = np.zeros((DP, M + HB), dtype=np.float16)
    xtp[:D, HB:] = X.T.astype(np.float16)

    b1m = np.asarray(inputs["b1m"], dtype=np.float32)
    b1p = np.asarray(inputs["b1p"], dtype=np.float32)
    bias_all = np.zeros((128, 4), dtype=np.float32)
    bias_all[:, 0] = b1m[:H1]
    bias_all[:H2, 1] = b1m[H1:]
    bias_all[:, 2] = b1p[:H1]
    bias_all[:H2, 3] = b1p[H1:]

    w2m = np.asarray(inputs["w2m"], dtype=np.float32)
    w2p = np.asarray(inputs["w2p"], dtype=np.float32)
    w2_all = np.zeros((128, 4), dtype=np.float16)
    w2_all[:, 0] = w2m[:H1].astype(np.float16)
    w2_all[:H2, 1] = w2m[H1:].astype(np.float16)
    w2_all[:, 2] = w2p[:H1].astype(np.float16)
    for g in range(4):
        w2_all[32 * g:32 * g + H2, 3] = w2p[H1:].astype(np.float16)

    shared = {
        "w1a": padD(W1a),
        "w1b": padD(W1b),
        "w1c": padD(W1c),
        "w1m": padD(np.asarray(inputs["w1m"], dtype=np.float32)),
        "bias_all": bias_all,
        "w2_all": w2_all,
        "idn": np.eye(128, dtype=np.float16),
    }

    b2m = float(np.asarray(inputs["b2m"]).reshape(-1)[0])
    b2p = float(np.asarray(inputs["b2p"]).reshape(-1)[0])
    in_maps = []
    for c in range(NCORES):
        r0 = MC * c
        xt_c = np.ascontiguousarray(xtp[:, r0:r0 + W])
        mmul = np.ones((MC, K + 1), dtype=np.float32)
        madd = np.full((MC, K + 1), np.float32(b2p + 2.0 * b2m), dtype=np.float32)
        mmul[:, K] = 0.0
        madd[:, K] = 0.0
        if c == 0:
            for i in range(min(K, MC)):
                mmul[i, :K - i] = 0.0
                madd[i, :K - i] = np.float32(-1e9)
        mm_ma = np.zeros((128, 4 * (K + 1)), dtype=np.float32)
        for mb in range(2):
            mm_ma[:, (K + 1) * mb:(K + 1) * (mb + 1)] = \
                mmul[128 * mb:128 * (mb + 1)]
            mm_ma[:, (K + 1) * (2 + mb):(K + 1) * (3 + mb)] = \
                madd[128 * mb:128 * (mb + 1)]
        in_maps.append({"xt": xt_c, "mm_ma": mm_ma, **shared})
    return in_maps


def _get_nc(inputs):
    if "nc" not in _cache:
        _cache["nc"] = _build()
    return _cache["nc"]


def _run(inputs, trace=False):
    assert int(np.asarray(inputs["K"])) == K
    nc = _get_nc(inputs)
    in_maps = _prep_inputs(inputs)
    res = run_bass_kernel_spmd(nc, in_maps, list(range(NCORES)), trace=trace)
    out = np.concatenate([res.results[c]["out"] for c in range(NCORES)], axis=0)
    return out.astype(np.float32), res


def kernel(**inputs) -> np.ndarray:
    out, _ = _run(inputs, trace=False)
    return out


# revision 19
# speedup vs baseline: 1.4832x; 1.0438x over previous
"""CorefScore kernel for 8 Trainium2 NeuronCores.

Shards the mention axis M=2048 across 8 cores (256 owned mentions plus a
64-row halo of preceding mentions). The banded pairwise MLP runs in
"superrounds" of up to 4 rounds (2 deltas each): the DVE emits one batched
shifted-product tensor_tensor per d-tile per superround ([128, 8*256] fp16),
the PE contracts them with W1c in fp16 (fp32 PSUM); the Ya + shift(Yb) terms
are injected straight into PSUM via identity matmuls with strided moving
operands (no DVE merge for the 128-wide h half). The 22-wide h2 half of all
4 rounds is packed into one PSUM bank at partition offsets 0/32/64/96 via
col-tiled matmuls that execute concurrently on disjoint PE column strips.
Pair scores (w2p contraction) use diagonal 32x32 tiles, also packed 4 rounds
per PSUM bank, and are respread/transposed per superround so only a tiny
tail remains after the last round. Inputs load via a handful of large DMAs
split across the Sync and ScalarE HWDGE queues.
"""

import os
import sys

import numpy as np

for _p in ("/opt/trn_rl_repo", "/opt/pypackages"):
    if os.path.isdir(_p) and _p not in sys.path:
        sys.path.append(_p)

import concourse.bacc as bacc
import concourse.bass as bass
import concourse.mybir as mybir
import concourse.tile as tile
from concourse.ap import AP
from concourse.bass_utils import run_bass_kernel_spmd

F16 = mybir.dt.float16
F32 = mybir.dt.float32
AF = mybir.ActivationFunctionType

M, D, H, K = 2048, 900, 150, 50
NCORES = 8
MC = M // NCORES          # owned mentions per core
HB = 64                   # halo columns (>= K)
W = MC + HB               # X^T window width per core
DP = 1024                 # padded feature dim (8 tiles of 128)
NDT = DP // 128           # number of d tiles
H1, H2 = 128, H - 128     # h split
# superround sizes (rounds of 2 deltas each); sum = 25 rounds = 50 deltas
SRS = [2, 4, 4, 4, 4, 4, 3]

_cache = {}


def _ap3(t_ap, p_lo, p_n, off, dims, pstep=1):
    """3-D free-dim view of a tile AP: partitions [p_lo, p_lo+p_n*pstep) with
    partition step pstep, free offset `off` elements, free dims."""
    b = t_ap[p_lo:p_lo + 1, 0:1]
    pstride = b.ap[0][0]
    return AP(b.tensor, b.offset + off,
              [[pstride * pstep, p_n]] + [list(d) for d in dims])


def _build():
    nc = bacc.Bacc("TRN2", target_bir_lowering=False, debug=False)

    xt_d = nc.dram_tensor("xt", [DP, W], F16, kind="ExternalInput").ap()
    w1a_d = nc.dram_tensor("w1a", [DP, H], F16, kind="ExternalInput").ap()
    w1b_d = nc.dram_tensor("w1b", [DP, H], F16, kind="ExternalInput").ap()
    w1c_d = nc.dram_tensor("w1c", [DP, H], F16, kind="ExternalInput").ap()
    w1m_d = nc.dram_tensor("w1m", [DP, H], F16, kind="ExternalInput").ap()
    bias_d = nc.dram_tensor("bias_all", [128, 4], F32, kind="ExternalInput").ap()
    w2_d = nc.dram_tensor("w2_all", [128, 4], F16, kind="ExternalInput").ap()
    idn_d = nc.dram_tensor("idn", [128, 128], F16, kind="ExternalInput").ap()
    mm_d = nc.dram_tensor("mm_ma", [128, 4 * (K + 1)], F32,
                          kind="ExternalInput").ap()
    out_d = nc.dram_tensor("out", [MC, K + 1], F32, kind="ExternalOutput").ap()

    def dma3(eng, dst_tile, src_dram, cols, t0=0, t1=NDT):
        """One DMA loading d-tiles [t0, t1) of [DP, cols] dram into a
        [128, NDT*cols] tile at the matching free offset."""
        src = AP(src_dram.tensor, src_dram.offset + 128 * cols * t0,
                 [[cols, 128], [128 * cols, t1 - t0], [1, cols]])
        eng.dma_start(out=dst_tile[:, cols * t0:cols * t1], in_=src)

    with tile.TileContext(nc) as tc:
        with (
            tc.tile_pool(name="cp", bufs=1) as cp,
            tc.tile_pool(name="wp", bufs=2) as wp,
            tc.tile_pool(name="pp", bufs=2, space="PSUM") as pp,
        ):
            # ---- input loads: sync queue (xt split across both queues) ----
            xt = cp.tile([128, NDT * W], F16, tag="xt")
            dma3(nc.sync, xt, xt_d, W, 0, NDT // 2)
            dma3(nc.scalar, xt, xt_d, W, NDT // 2, NDT)
            bias = cp.tile([128, 4], F32, tag="bias")
            nc.sync.dma_start(out=bias[:], in_=bias_d[:])
            idn = cp.tile([128, 128], F16, tag="idn")
            nc.sync.dma_start(out=idn[:], in_=idn_d[:])
            w2 = cp.tile([128, 4], F16, tag="w2")
            nc.sync.dma_start(out=w2[:], in_=w2_d[:])
            mm_ma = cp.tile([128, 4 * (K + 1)], F32, tag="mm_ma")
            nc.sync.dma_start(out=mm_ma[:], in_=mm_d[:])
            # ---- input loads: scalar (Act) HWDGE queue ----
            w1a = cp.tile([128, NDT * H], F16, tag="w1a")
            dma3(nc.scalar, w1a, w1a_d, H)
            w1b = cp.tile([128, NDT * H], F16, tag="w1b")
            dma3(nc.scalar, w1b, w1b_d, H)
            w1m = cp.tile([128, NDT * H], F16, tag="w1m")
            dma3(nc.scalar, w1m, w1m_d, H)
            w1c = cp.tile([128, NDT * H], F16, tag="w1c")
            dma3(nc.scalar, w1c, w1c_d, H)

            def wsl(wt, t, ho, hn):  # stationary slice of a weight tile
                return wt[:, H * t + ho:H * t + ho + hn]

            # ---- preamble MLPs: Ya (owned, +b1p), Yb (window), mention ----
            psya1 = pp.tile([H1, MC], F32, tag="ah1")
            psya2 = pp.tile([H2, MC], F32, tag="ah1")
            psyb1 = pp.tile([H1, W], F32, tag="ah2")
            psyb2 = pp.tile([H2, W], F32, tag="ah2")
            psm1 = pp.tile([H1, W], F32, tag="psp")
            psm2 = pp.tile([H2, W], F32, tag="psp")
            for t in range(NDT):
                nc.tensor.matmul(psya1[:], wsl(w1a, t, 0, H1),
                                 xt[:, W * t + HB:W * (t + 1)],
                                 start=(t == 0), stop=(t == NDT - 1))
            for t in range(NDT):
                nc.tensor.matmul(psya2[:], wsl(w1a, t, H1, H2),
                                 xt[:, W * t + HB:W * (t + 1)],
                                 start=(t == 0), stop=(t == NDT - 1))
            for t in range(NDT):
                nc.tensor.matmul(psyb1[:], wsl(w1b, t, 0, H1),
                                 xt[:, W * t:W * (t + 1)],
                                 start=(t == 0), stop=(t == NDT - 1))
            for t in range(NDT):
                nc.tensor.matmul(psyb2[:], wsl(w1b, t, H1, H2),
                                 xt[:, W * t:W * (t + 1)],
                                 start=(t == 0), stop=(t == NDT - 1))
            for t in range(NDT):
                nc.tensor.matmul(psm1[:], wsl(w1m, t, 0, H1),
                                 xt[:, W * t:W * (t + 1)],
                                 start=(t == 0), stop=(t == NDT - 1))
            for t in range(NDT):
                nc.tensor.matmul(psm2[:], wsl(w1m, t, H1, H2),
                                 xt[:, W * t:W * (t + 1)],
                                 start=(t == 0), stop=(t == NDT - 1))

            ya1 = cp.tile([H1, MC], F16, tag="ya1")
            nc.scalar.activation(ya1[:], psya1[:], AF.Identity, bias=bias[:, 2:3])
            ya2 = cp.tile([H2, MC], F16, tag="ya2")
            nc.scalar.activation(ya2[:], psya2[:], AF.Identity,
                                 bias=bias[0:H2, 3:4])
            yb1 = cp.tile([H1, W], F16, tag="yb1")
            nc.scalar.copy(yb1[:], psyb1[:])
            yb2 = cp.tile([H2, W], F16, tag="yb2")
            nc.scalar.copy(yb2[:], psyb2[:])
            ma1 = cp.tile([H1, W], F16, tag="ma1")
            nc.scalar.activation(ma1[:], psm1[:], AF.Relu, bias=bias[:, 0:1])
            ma2 = cp.tile([H2, W], F16, tag="ma2")
            nc.scalar.activation(ma2[:], psm2[:], AF.Relu, bias=bias[0:H2, 1:2])

            # mention score row over the window
            psme = pp.tile([1, W], F32, tag="pre")
            nc.tensor.matmul(psme[:], w2[:, 0:1], ma1[:], start=True, stop=False)
            nc.tensor.matmul(psme[:], w2[0:H2, 1:2], ma2[:], start=False,
                             stop=True)
            # row 0 = mention scores; rows 1..31 stay zero so the e0-column
            # identity-inject matmul (32-wide contraction) picks row 0 only
            ment_row = cp.tile([32, W], F16, tag="ment_row")
            nc.vector.memset(ment_row[:], 0.0)
            nc.scalar.copy(ment_row[0:1, :], psme[:])
            # ment as per-partition columns for the owned 2x128 mention blocks
            ment_col = []
            for mb in range(2):
                pst = pp.tile([128, 1], F16, tag="pre")
                nc.tensor.transpose(pst[:],
                                    ment_row[0:1, HB + 128 * mb:HB + 128 * (mb + 1)],
                                    idn[0:1, 0:1])
                mcol = cp.tile([128, 1], F32, tag=f"mcol{mb}")
                nc.scalar.copy(mcol[:], pst[:])
                ment_col.append(mcol)

            scm = []
            for mb in range(2):
                s = cp.tile([128, K + 1], F32, tag=f"scm{mb}")
                nc.vector.memset(s[:], 0.0)
                scm.append(s)

            # ---- DVE: batched shifted products per superround ----
            # SR s covers rounds r0..r0+R-1; delta of column group j (0..2R-1)
            # is d0 - j with d0 = K - 2*r0; product col j*MC+m multiplies
            # X^T[., m] (owned) by X^T[., m - (d0 - j)].
            r0s = []
            acc = 0
            for R in SRS:
                r0s.append(acc)
                acc += R

            def emit_products(s):
                R = SRS[s]
                d0 = K - 2 * r0s[s]
                pts = []
                for t in range(NDT):
                    pt = wp.tile([128, 2 * R * MC], F16, tag=f"pt{t}",
                                 padded_shape=[128, 8 * MC])
                    nc.vector.tensor_tensor(
                        _ap3(pt[:], 0, 128, 0, [(MC, 2 * R), (1, MC)]),
                        _ap3(xt[:], 0, 128, W * t + HB, [(0, 2 * R), (1, MC)]),
                        _ap3(xt[:], 0, 128, W * t + HB - d0, [(1, 2 * R), (1, MC)]),
                        mybir.AluOpType.mult)
                    pts.append(pt)
                return pts

            def emit_c2(s):
                R = SRS[s]
                d0 = K - 2 * r0s[s]
                c2 = wp.tile([H2, 2 * R * MC], F16, tag="c2",
                             padded_shape=[128, 8 * MC])
                nc.vector.tensor_tensor(
                    _ap3(c2[:], 0, H2, 0, [(MC, 2 * R), (1, MC)]),
                    _ap3(ya2[:], 0, H2, 0, [(0, 2 * R), (1, MC)]),
                    _ap3(yb2[:], 0, H2, HB - d0, [(1, 2 * R), (1, MC)]),
                    mybir.AluOpType.add)
                return c2

            def emit_c1(s):
                R = SRS[s]
                d0 = K - 2 * r0s[s]
                c1 = wp.tile([H1, 2 * R * MC], F16, tag="c1",
                             padded_shape=[128, 8 * MC])
                nc.vector.tensor_tensor(
                    _ap3(c1[:], 0, H1, 0, [(MC, 2 * R), (1, MC)]),
                    _ap3(ya1[:], 0, H1, 0, [(0, 2 * R), (1, MC)]),
                    _ap3(yb1[:], 0, H1, HB - d0, [(1, 2 * R), (1, MC)]),
                    mybir.AluOpType.add)
                return c1

            pts_q = {0: emit_products(0)}
            c1_q = {0: emit_c1(0)}
            c2_q = {0: emit_c2(0)}
            pts_q[1] = emit_products(1)

            # ---- superround loop state ----
            a1_ring = []      # (sr, g) -> a1 tile, kept one SR back
            prev = None       # state of SR s-1 for deferred pair work

            pairK_pool = wp   # per-SR respread tiles

            def emit_pairs_for(state):
                """PE pair matmuls + evac + respread + transposes for SR s-1."""
                s, R, a1s, a2xs = state
                d0 = K - 2 * r0s[s]
                psp = pp.tile([97, MC * 2], F32, tag="psp")
                for g in range(R):
                    # ment_j first (start=True): a strided-moving matmul with
                    # start=False wedges the exec unit, so it opens the group
                    nc.tensor.matmul(
                        psp[32 * g:32 * g + 1, :], idn[0:32, 0:1],
                        _ap3(ment_row[:], 0, 32, HB - (d0 - 2 * g),
                             [(1, 2), (1, MC)]),
                        start=True, stop=False, tile_position=(0, 32 * g))
                    nc.tensor.matmul(psp[32 * g:32 * g + 1, :], w2[:, 2:3],
                                     a1s[g][:], start=False, stop=False,
                                     tile_position=(0, 32 * g))
                    nc.tensor.matmul(psp[32 * g:32 * g + 1, :],
                                     w2[32 * g:32 * g + H2, 3:4],
                                     a2xs[32 * g:32 * g + H2, :],
                                     start=False, stop=True,
                                     tile_position=(32 * g, 32 * g))
                pair_sb = wp.tile([97, MC * 2], F16, tag="pair_sb")
                nc.scalar.copy(pair_sb[:], psp[:])
                pairK = pairK_pool.tile([2 * R, MC], F16, tag="pairK",
                                        padded_shape=[128, MC])
                nc.sync.dma_start(
                    out=pairK[:],
                    in_=_ap3(pair_sb[:], 0, R, 0, [(MC, 2), (1, MC)], pstep=32))
                return pairK

            def emit_respread_tail(state, pairK):
                """Transposes + scM chunk evac for SR s-1 (after its DMA)."""
                s, R, a1s, a2xs = state
                k0 = 2 * r0s[s]
                cn = 2 * R
                for mb in range(2):
                    ptr = pp.tile([128, cn], F16, tag="pre",
                                  padded_shape=[128, 8])
                    nc.tensor.transpose(ptr[:], pairK[0:cn, 128 * mb:128 * (mb + 1)],
                                        idn[0:cn, 0:cn])
                    nc.scalar.activation(scm[mb][:, k0:k0 + cn], ptr[:],
                                         AF.Identity, bias=ment_col[mb][:])

            for s, R in enumerate(SRS):
                r0 = r0s[s]
                d0 = K - 2 * r0
                pts = pts_q.pop(s)
                c1 = c1_q.pop(s)
                c2 = c2_q.pop(s)

                # deferred pair matmuls for the previous superround
                pairK_prev = emit_pairs_for(prev) if prev is not None else None

                # per-round h1 streams (+ Ya / shifted-Yb injections)
                a1s = []
                for g in range(R):
                    ps1 = pp.tile([H1, 2 * MC], F32, tag="ah1")
                    for t in range(NDT):
                        nc.tensor.matmul(ps1[:], wsl(w1c, t, 0, H1),
                                         pts[t][:, 2 * g * MC:(2 * g + 2) * MC],
                                         start=(t == 0), stop=False)
                    nc.tensor.matmul(
                        ps1[:], idn[:], c1[:, 2 * g * MC:(2 * g + 2) * MC],
                        start=False, stop=True)
                    a1 = wp.tile([H1, 2 * MC], F16, tag="a1", bufs=8)
                    nc.scalar.activation(a1[:], ps1[:], AF.Relu)
                    a1s.append(a1)

                # packed h2: col-tiled matmuls, 4 rounds -> one PSUM bank
                hp = 32 * (R - 1) + H2
                ps2 = pp.tile([hp, 2 * MC], F32, tag="ah2")
                for t in range(NDT):
                    for g in range(R):
                        nc.tensor.matmul(ps2[32 * g:32 * g + H2, :],
                                         wsl(w1c, t, H1, H2),
                                         pts[t][:, 2 * g * MC:(2 * g + 2) * MC],
                                         start=(t == 0), stop=False,
                                         tile_position=(0, 32 * g))
                for g in range(R):
                    nc.tensor.matmul(ps2[32 * g:32 * g + H2, :],
                                     idn[0:H2, 0:H2],
                                     c2[0:H2, 2 * g * MC:(2 * g + 2) * MC],
                                     start=False, stop=True,
                                     tile_position=(0, 32 * g))
                a2x = wp.tile([hp, 2 * MC], F16, tag="a2x",
                              padded_shape=[128, 2 * MC])
                nc.scalar.activation(a2x[:], ps2[:], AF.Relu)

                # respread tail for SR s-1 (its DMA has landed by now)
                if prev is not None:
                    emit_respread_tail(prev, pairK_prev)

                # prefetch DVE work for s+2 / c2 for s+1
                if s + 1 < len(SRS):
                    c1_q[s + 1] = emit_c1(s + 1)
                    c2_q[s + 1] = emit_c2(s + 1)
                if s + 2 < len(SRS):
                    pts_q[s + 2] = emit_products(s + 2)

                prev = (s, R, a1s, a2x[:])

            # ---- epilogue: pairs of the last superround + final masking ----
            pairK_last = emit_pairs_for(prev)
            emit_respread_tail(prev, pairK_last)
            for mb in range(2):
                nc.vector.tensor_mul(scm[mb][:], scm[mb][:],
                                     mm_ma[:, (K + 1) * mb:(K + 1) * (mb + 1)])
                nc.vector.tensor_add(
                    scm[mb][:], scm[mb][:],
                    mm_ma[:, (K + 1) * (2 + mb):(K + 1) * (3 + mb)])
                nc.sync.dma_start(out=out_d[128 * mb:128 * (mb + 1), :],
                                  in_=scm[mb][:])

    nc.compile()
    return nc


def _prep_inputs(inputs):
    X = np.ascontiguousarray(inputs["mention_reprs"], dtype=np.float32)
    assert X.shape == (M, D)
    w1p = np.asarray(inputs["w1p"], dtype=np.float32)
    W1a, W1b, W1c = w1p[:D], w1p[D:2 * D], w1p[2 * D:]

    def padD(w):  # [D, H] -> [DP, H] fp16
        out = np.zeros((DP, H), dtype=np.float16)
        out[:D] = w.astype(np.float16)
        return out

    xtp = np.zeros((DP, M + HB), dtype=np.float16)
    xtp[:D, HB:] = X.T.astype(np.float16)

    b1m = np.asarray(inputs["b1m"], dtype=np.float32)
    b1p = np.asarray(inputs["b1p"], dtype=np.float32)
    bias_all = np.zeros((128, 4), dtype=np.float32)
    bias_all[:, 0] = b1m[:H1]
    bias_all[:H2, 1] = b1m[H1:]
    bias_all[:, 2] = b1p[:H1]
    bias_all[:H2, 3] = b1p[H1:]

    w2m = np.asarray(inputs["w2m"], dtype=np.float32)
    w2p = np.asarray(inputs["w2p"], dtype=np.float32)
    w2_all = np.zeros((128, 4), dtype=np.float16)
    w2_all[:, 0] = w2m[:H1].astype(np.float16)
    w2_all[:H2, 1] = w2m[H1:].astype(np.float16)
    w2_all[:, 2] = w2p[:H1].astype(np.float16)
    for g in range(4):
        w2_all[32 * g:32 * g + H2, 3] = w2p[H1:].astype(np.float16)

    shared = {
        "w1a": padD(W1a),
        "w1b": padD(W1b),
        "w1c": padD(W1c),
        "w1m": padD(np.asarray(inputs["w1m"], dtype=np.float32)),
        "bias_all": bias_all,
        "w2_all": w2_all,
        "idn": np.eye(128, dtype=np.float16),
    }

    b2m = float(np.asarray(inputs["b2m"]).reshape(-1)[0])
    b2p = float(np.asarray(inputs["b2p"]).reshape(-1)[0])
    in_maps = []
    for c in range(NCORES):
        r0 = MC * c
        xt_c = np.ascontiguousarray(xtp[:, r0:r0 + W])
        mmul = np.ones((MC, K + 1), dtype=np.float32)
        madd = np.full((MC, K + 1), np.float32(b2p + 2.0 * b2m), dtype=np.float32)
        mmul[:, K] = 0.0
        madd[:, K] = 0.0
        if c == 0:
            for i in range(min(K, MC)):
                mmul[i, :K - i] = 0.0
                madd[i, :K - i] = np.float32(-1e9)
        mm_ma = np.zeros((128, 4 * (K + 1)), dtype=np.float32)
        for mb in range(2):
            mm_ma[:, (K + 1) * mb:(K + 1) * (mb + 1)] = \
                mmul[128 * mb:128 * (mb + 1)]
            mm_ma[:, (K + 1) * (2 + mb):(K + 1) * (3 + mb)] = \
                madd[128 * mb:128 * (mb + 1)]
        in_maps.append({"xt": xt_c, "mm_ma": mm_ma, **shared})
    return in_maps


def _get_nc(inputs):
    if "nc" not in _cache:
        _cache["nc"] = _build()
    return _cache["nc"]


def _run(inputs, trace=False):
    assert int(np.asarray(inputs["K"])) == K
    nc = _get_nc(inputs)
    in_maps = _prep_inputs(inputs)
    res = run_bass_kernel_spmd(nc, in_maps, list(range(NCORES)), trace=trace)
    out = np.concatenate([res.results[c]["out"] for c in range(NCORES)], axis=0)
    return out.astype(np.float32), res


def kernel(**inputs) -> np.ndarray:
    out, _ = _run(inputs, trace=False)
    return out


# revision 22
# speedup vs baseline: 1.4862x; 1.0020x over previous
"""CorefScore kernel for 8 Trainium2 NeuronCores.

Shards the mention axis M=2048 across 8 cores (256 owned mentions plus a
64-row halo of preceding mentions). The banded pairwise MLP runs in
"superrounds" of up to 4 rounds (2 deltas each): the DVE emits one batched
shifted-product tensor_tensor per d-tile per superround ([128, 8*256] fp16),
the PE contracts them with W1c in fp16 (fp32 PSUM); the Ya + shift(Yb) terms
are injected straight into PSUM via identity matmuls with strided moving
operands (no DVE merge for the 128-wide h half). The 22-wide h2 half of all
4 rounds is packed into one PSUM bank at partition offsets 0/32/64/96 via
col-tiled matmuls that execute concurrently on disjoint PE column strips.
Pair scores (w2p contraction) use diagonal 32x32 tiles, also packed 4 rounds
per PSUM bank, and are respread/transposed per superround so only a tiny
tail remains after the last round. Inputs load via a handful of large DMAs
split across the Sync and ScalarE HWDGE queues.
"""

import os
import sys

import numpy as np

for _p in ("/opt/trn_rl_repo", "/opt/pypackages"):
    if os.path.isdir(_p) and _p not in sys.path:
        sys.path.append(_p)

import concourse.bacc as bacc
import concourse.bass as bass
import concourse.mybir as mybir
import concourse.tile as tile
from concourse.ap import AP
from concourse.bass_utils import run_bass_kernel_spmd

F16 = mybir.dt.float16
F32 = mybir.dt.float32
AF = mybir.ActivationFunctionType

M, D, H, K = 2048, 900, 150, 50
NCORES = 8
MC = M // NCORES          # owned mentions per core
HB = 64                   # halo columns (>= K)
W = MC + HB               # X^T window width per core
DP = 1024                 # padded feature dim (8 tiles of 128)
NDT = DP // 128           # number of d tiles
H1, H2 = 128, H - 128     # h split
# superround sizes (rounds of 2 deltas each); sum = 25 rounds = 50 deltas;
# last kept smallest so the epilogue pair/respread chain is short
SRS = [3, 4, 4, 4, 4, 4, 2]

_cache = {}


def _ap3(t_ap, p_lo, p_n, off, dims, pstep=1):
    """3-D free-dim view of a tile AP: partitions [p_lo, p_lo+p_n*pstep) with
    partition step pstep, free offset `off` elements, free dims."""
    b = t_ap[p_lo:p_lo + 1, 0:1]
    pstride = b.ap[0][0]
    return AP(b.tensor, b.offset + off,
              [[pstride * pstep, p_n]] + [list(d) for d in dims])


def _build():
    nc = bacc.Bacc("TRN2", target_bir_lowering=False, debug=False)

    xt_d = nc.dram_tensor("xt", [DP, W], F16, kind="ExternalInput").ap()
    w1a_d = nc.dram_tensor("w1a", [DP, H], F16, kind="ExternalInput").ap()
    w1b_d = nc.dram_tensor("w1b", [DP, H], F16, kind="ExternalInput").ap()
    w1c_d = nc.dram_tensor("w1c", [DP, H], F16, kind="ExternalInput").ap()
    w1m_d = nc.dram_tensor("w1m", [DP, H], F16, kind="ExternalInput").ap()
    bias_d = nc.dram_tensor("bias_all", [128, 4], F32, kind="ExternalInput").ap()
    w2_d = nc.dram_tensor("w2_all", [128, 4], F16, kind="ExternalInput").ap()
    idn_d = nc.dram_tensor("idn", [128, 128], F16, kind="ExternalInput").ap()
    mm_d = nc.dram_tensor("mm_ma", [128, 4 * (K + 1)], F32,
                          kind="ExternalInput").ap()
    out_d = nc.dram_tensor("out", [MC, K + 1], F32, kind="ExternalOutput").ap()

    def dma3(eng, dst_tile, src_dram, cols, t0=0, t1=NDT):
        """One DMA loading d-tiles [t0, t1) of [DP, cols] dram into a
        [128, NDT*cols] tile at the matching free offset."""
        src = AP(src_dram.tensor, src_dram.offset + 128 * cols * t0,
                 [[cols, 128], [128 * cols, t1 - t0], [1, cols]])
        eng.dma_start(out=dst_tile[:, cols * t0:cols * t1], in_=src)

    with tile.TileContext(nc) as tc:
        with (
            tc.tile_pool(name="cp", bufs=1) as cp,
            tc.tile_pool(name="wp", bufs=2) as wp,
            tc.tile_pool(name="pp", bufs=2, space="PSUM") as pp,
        ):
            # ---- input loads: sync queue (xt split across both queues;
            # w1a first on the scalar queue so the Ya stream starts early) ----
            xt = cp.tile([128, NDT * W], F16, tag="xt")
            dma3(nc.sync, xt, xt_d, W, 0, NDT // 2)
            w1a = cp.tile([128, NDT * H], F16, tag="w1a")
            dma3(nc.scalar, w1a, w1a_d, H)
            dma3(nc.scalar, xt, xt_d, W, NDT // 2, NDT)
            bias = cp.tile([128, 4], F32, tag="bias")
            nc.sync.dma_start(out=bias[:], in_=bias_d[:])
            idn = cp.tile([128, 128], F16, tag="idn")
            nc.sync.dma_start(out=idn[:], in_=idn_d[:])
            w2 = cp.tile([128, 4], F16, tag="w2")
            nc.sync.dma_start(out=w2[:], in_=w2_d[:])
            mm_ma = cp.tile([128, 4 * (K + 1)], F32, tag="mm_ma")
            nc.sync.dma_start(out=mm_ma[:], in_=mm_d[:])
            # ---- input loads: scalar (Act) HWDGE queue ----
            w1b = cp.tile([128, NDT * H], F16, tag="w1b")
            dma3(nc.scalar, w1b, w1b_d, H)
            w1m = cp.tile([128, NDT * H], F16, tag="w1m")
            dma3(nc.scalar, w1m, w1m_d, H)
            w1c = cp.tile([128, NDT * H], F16, tag="w1c")
            dma3(nc.scalar, w1c, w1c_d, H)

            def wsl(wt, t, ho, hn):  # stationary slice of a weight tile
                return wt[:, H * t + ho:H * t + ho + hn]

            # ---- preamble MLPs: Ya (owned, +b1p), Yb (window), mention ----
            psya1 = pp.tile([H1, MC], F32, tag="ah1")
            psya2 = pp.tile([H2, MC], F32, tag="ah1")
            psyb1 = pp.tile([H1, W], F32, tag="ah2")
            psyb2 = pp.tile([H2, W], F32, tag="ah2")
            psm1 = pp.tile([H1, W], F32, tag="psp")
            psm2 = pp.tile([H2, W], F32, tag="psp")
            for t in range(NDT):
                nc.tensor.matmul(psya1[:], wsl(w1a, t, 0, H1),
                                 xt[:, W * t + HB:W * (t + 1)],
                                 start=(t == 0), stop=(t == NDT - 1))
            for t in range(NDT):
                nc.tensor.matmul(psya2[:], wsl(w1a, t, H1, H2),
                                 xt[:, W * t + HB:W * (t + 1)],
                                 start=(t == 0), stop=(t == NDT - 1))
            for t in range(NDT):
                nc.tensor.matmul(psyb1[:], wsl(w1b, t, 0, H1),
                                 xt[:, W * t:W * (t + 1)],
                                 start=(t == 0), stop=(t == NDT - 1))
            for t in range(NDT):
                nc.tensor.matmul(psyb2[:], wsl(w1b, t, H1, H2),
                                 xt[:, W * t:W * (t + 1)],
                                 start=(t == 0), stop=(t == NDT - 1))
            for t in range(NDT):
                nc.tensor.matmul(psm1[:], wsl(w1m, t, 0, H1),
                                 xt[:, W * t:W * (t + 1)],
                                 start=(t == 0), stop=(t == NDT - 1))
            for t in range(NDT):
                nc.tensor.matmul(psm2[:], wsl(w1m, t, H1, H2),
                                 xt[:, W * t:W * (t + 1)],
                                 start=(t == 0), stop=(t == NDT - 1))

            ya1 = cp.tile([H1, MC], F16, tag="ya1")
            nc.scalar.activation(ya1[:], psya1[:], AF.Identity, bias=bias[:, 2:3])
            ya2 = cp.tile([H2, MC], F16, tag="ya2")
            nc.scalar.activation(ya2[:], psya2[:], AF.Identity,
                                 bias=bias[0:H2, 3:4])
            yb1 = cp.tile([H1, W], F16, tag="yb1")
            nc.scalar.copy(yb1[:], psyb1[:])
            yb2 = cp.tile([H2, W], F16, tag="yb2")
            nc.scalar.copy(yb2[:], psyb2[:])
            ma1 = cp.tile([H1, W], F16, tag="ma1")
            nc.scalar.activation(ma1[:], psm1[:], AF.Relu, bias=bias[:, 0:1])
            ma2 = cp.tile([H2, W], F16, tag="ma2")
            nc.scalar.activation(ma2[:], psm2[:], AF.Relu, bias=bias[0:H2, 1:2])

            # mention score row over the window
            psme = pp.tile([1, W], F32, tag="pre")
            nc.tensor.matmul(psme[:], w2[:, 0:1], ma1[:], start=True, stop=False)
            nc.tensor.matmul(psme[:], w2[0:H2, 1:2], ma2[:], start=False,
                             stop=True)
            # row 0 = mention scores; rows 1..31 stay zero so the e0-column
            # identity-inject matmul (32-wide contraction) picks row 0 only
            ment_row = cp.tile([32, W], F16, tag="ment_row")
            nc.vector.memset(ment_row[:], 0.0)
            nc.scalar.copy(ment_row[0:1, :], psme[:])
            # ment as per-partition columns for the owned 2x128 mention blocks
            ment_col = []
            for mb in range(2):
                pst = pp.tile([128, 1], F16, tag="pre")
                nc.tensor.transpose(pst[:],
                                    ment_row[0:1, HB + 128 * mb:HB + 128 * (mb + 1)],
                                    idn[0:1, 0:1])
                mcol = cp.tile([128, 1], F32, tag=f"mcol{mb}")
                nc.scalar.copy(mcol[:], pst[:])
                ment_col.append(mcol)

            scm = []
            for mb in range(2):
                s = cp.tile([128, K + 1], F32, tag=f"scm{mb}")
                nc.vector.memset(s[:], 0.0)
                scm.append(s)

            # ---- DVE: batched shifted products per superround ----
            # SR s covers rounds r0..r0+R-1; delta of column group j (0..2R-1)
            # is d0 - j with d0 = K - 2*r0; product col j*MC+m multiplies
            # X^T[., m] (owned) by X^T[., m - (d0 - j)].
            r0s = []
            acc = 0
            for R in SRS:
                r0s.append(acc)
                acc += R

            def emit_products(s):
                R = SRS[s]
                d0 = K - 2 * r0s[s]
                pts = []
                for t in range(NDT):
                    pt = wp.tile([128, 2 * R * MC], F16, tag=f"pt{t}",
                                 padded_shape=[128, 8 * MC])
                    nc.vector.tensor_tensor(
                        _ap3(pt[:], 0, 128, 0, [(MC, 2 * R), (1, MC)]),
                        _ap3(xt[:], 0, 128, W * t + HB, [(0, 2 * R), (1, MC)]),
                        _ap3(xt[:], 0, 128, W * t + HB - d0, [(1, 2 * R), (1, MC)]),
                        mybir.AluOpType.mult)
                    pts.append(pt)
                return pts

            def emit_c2(s):
                R = SRS[s]
                d0 = K - 2 * r0s[s]
                c2 = wp.tile([H2, 2 * R * MC], F16, tag="c2",
                             padded_shape=[128, 8 * MC])
                nc.vector.tensor_tensor(
                    _ap3(c2[:], 0, H2, 0, [(MC, 2 * R), (1, MC)]),
                    _ap3(ya2[:], 0, H2, 0, [(0, 2 * R), (1, MC)]),
                    _ap3(yb2[:], 0, H2, HB - d0, [(1, 2 * R), (1, MC)]),
                    mybir.AluOpType.add)
                return c2

            def emit_c1(s):
                R = SRS[s]
                d0 = K - 2 * r0s[s]
                c1 = wp.tile([H1, 2 * R * MC], F16, tag="c1",
                             padded_shape=[128, 8 * MC])
                nc.vector.tensor_tensor(
                    _ap3(c1[:], 0, H1, 0, [(MC, 2 * R), (1, MC)]),
                    _ap3(ya1[:], 0, H1, 0, [(0, 2 * R), (1, MC)]),
                    _ap3(yb1[:], 0, H1, HB - d0, [(1, 2 * R), (1, MC)]),
                    mybir.AluOpType.add)
                return c1

            pts_q = {0: emit_products(0)}
            c1_q = {0: emit_c1(0)}
            c2_q = {0: emit_c2(0)}
            pts_q[1] = emit_products(1)

            # ---- superround loop state ----
            a1_ring = []      # (sr, g) -> a1 tile, kept one SR back
            prev = None       # state of SR s-1 for deferred pair work

            pairK_pool = wp   # per-SR respread tiles

            def emit_pairs_for(state):
                """PE pair matmuls + evac + respread + transposes for SR s-1."""
                s, R, a1s, a2xs = state
                d0 = K - 2 * r0s[s]
                psp = pp.tile([97, MC * 2], F32, tag="psp")
                for g in range(R):
                    # ment_j first (start=True): a strided-moving matmul with
                    # start=False wedges the exec unit, so it opens the group
                    nc.tensor.matmul(
                        psp[32 * g:32 * g + 1, :], idn[0:32, 0:1],
                        _ap3(ment_row[:], 0, 32, HB - (d0 - 2 * g),
                             [(1, 2), (1, MC)]),
                        start=True, stop=False, tile_position=(0, 32 * g))
                    nc.tensor.matmul(psp[32 * g:32 * g + 1, :], w2[:, 2:3],
                                     a1s[g][:], start=False, stop=False,
                                     tile_position=(0, 32 * g))
                    nc.tensor.matmul(psp[32 * g:32 * g + 1, :],
                                     w2[32 * g:32 * g + H2, 3:4],
                                     a2xs[32 * g:32 * g + H2, :],
                                     start=False, stop=True,
                                     tile_position=(32 * g, 32 * g))
                pair_sb = wp.tile([97, MC * 2], F16, tag="pair_sb")
                nc.scalar.copy(pair_sb[:], psp[:])
                pairK = pairK_pool.tile([2 * R, MC], F16, tag="pairK",
                                        padded_shape=[128, MC])
                nc.sync.dma_start(
                    out=pairK[:],
                    in_=_ap3(pair_sb[:], 0, R, 0, [(MC, 2), (1, MC)], pstep=32))
                return pairK

            def emit_respread_tail(state, pairK):
                """Transposes + scM chunk evac for SR s-1 (after its DMA)."""
                s, R, a1s, a2xs = state
                k0 = 2 * r0s[s]
                cn = 2 * R
                for mb in range(2):
                    ptr = pp.tile([128, cn], F16, tag="pre",
                                  padded_shape=[128, 8])
                    nc.tensor.transpose(ptr[:], pairK[0:cn, 128 * mb:128 * (mb + 1)],
                                        idn[0:cn, 0:cn])
                    nc.scalar.activation(scm[mb][:, k0:k0 + cn], ptr[:],
                                         AF.Identity, bias=ment_col[mb][:])

            for s, R in enumerate(SRS):
                r0 = r0s[s]
                d0 = K - 2 * r0
                pts = pts_q.pop(s)
                c1 = c1_q.pop(s)
                c2 = c2_q.pop(s)

                # deferred pair matmuls for the previous superround
                pairK_prev = emit_pairs_for(prev) if prev is not None else None

                # per-round h1 streams (+ Ya / shifted-Yb injections)
                a1s = []
                for g in range(R):
                    ps1 = pp.tile([H1, 2 * MC], F32, tag="ah1")
                    for t in range(NDT):
                        nc.tensor.matmul(ps1[:], wsl(w1c, t, 0, H1),
                                         pts[t][:, 2 * g * MC:(2 * g + 2) * MC],
                                         start=(t == 0), stop=False)
                    nc.tensor.matmul(
                        ps1[:], idn[:], c1[:, 2 * g * MC:(2 * g + 2) * MC],
                        start=False, stop=True)
                    a1 = wp.tile([H1, 2 * MC], F16, tag="a1", bufs=8)
                    nc.scalar.activation(a1[:], ps1[:], AF.Relu)
                    a1s.append(a1)

                # packed h2: col-tiled matmuls, 4 rounds -> one PSUM bank
                hp = 32 * (R - 1) + H2
                ps2 = pp.tile([hp, 2 * MC], F32, tag="ah2")
                for t in range(NDT):
                    for g in range(R):
                        nc.tensor.matmul(ps2[32 * g:32 * g + H2, :],
                                         wsl(w1c, t, H1, H2),
                                         pts[t][:, 2 * g * MC:(2 * g + 2) * MC],
                                         start=(t == 0), stop=False,
                                         tile_position=(0, 32 * g))
                for g in range(R):
                    nc.tensor.matmul(ps2[32 * g:32 * g + H2, :],
                                     idn[0:H2, 0:H2],
                                     c2[0:H2, 2 * g * MC:(2 * g + 2) * MC],
                                     start=False, stop=True,
                                     tile_position=(0, 32 * g))
                a2x = wp.tile([hp, 2 * MC], F16, tag="a2x",
                              padded_shape=[128, 2 * MC])
                nc.scalar.activation(a2x[:], ps2[:], AF.Relu)

                # respread tail for SR s-1 (its DMA has landed by now)
                if prev is not None:
                    emit_respread_tail(prev, pairK_prev)

                # prefetch DVE work for s+2 / c2 for s+1
                if s + 1 < len(SRS):
                    c1_q[s + 1] = emit_c1(s + 1)
                    c2_q[s + 1] = emit_c2(s + 1)
                if s + 2 < len(SRS):
                    pts_q[s + 2] = emit_products(s + 2)

                prev = (s, R, a1s, a2x[:])

            # ---- epilogue: pairs of the last superround + final masking ----
            pairK_last = emit_pairs_for(prev)
            emit_respread_tail(prev, pairK_last)
            for mb in range(2):
                nc.vector.tensor_mul(scm[mb][:], scm[mb][:],
                                     mm_ma[:, (K + 1) * mb:(K + 1) * (mb + 1)])
                nc.vector.tensor_add(
                    scm[mb][:], scm[mb][:],
                    mm_ma[:, (K + 1) * (2 + mb):(K + 1) * (3 + mb)])
                nc.sync.dma_start(out=out_d[128 * mb:128 * (mb + 1), :],
                                  in_=scm[mb][:])

    nc.compile()
    return nc


def _prep_inputs(inputs):
    X = np.ascontiguousarray(inputs["mention_reprs"], dtype=np.float32)
    assert X.shape == (M, D)
    w1p = np.asarray(inputs["w1p"], dtype=np.float32)
    W1a, W1b, W1c = w1p[:D], w1p[D:2 * D], w1p[2 * D:]

    def padD(w):  # [D, H] -> [DP, H] fp16
        out = np.zeros((DP, H), dtype=np.float16)
        out[:D] = w.astype(np.float16)
        return out

    xtp = np.zeros((DP, M + HB), dtype=np.float16)
    xtp[:D, HB:] = X.T.astype(np.float16)

    b1m = np.asarray(inputs["b1m"], dtype=np.float32)
    b1p = np.asarray(inputs["b1p"], dtype=np.float32)
    bias_all = np.zeros((128, 4), dtype=np.float32)
    bias_all[:, 0] = b1m[:H1]
    bias_all[:H2, 1] = b1m[H1:]
    bias_all[:, 2] = b1p[:H1]
    bias_all[:H2, 3] = b1p[H1:]

    w2m = np.asarray(inputs["w2m"], dtype=np.float32)
    w2p = np.asarray(inputs["w2p"], dtype=np.float32)
    w2_all = np.zeros((128, 4), dtype=np.float16)
    w2_all[:, 0] = w2m[:H1].astype(np.float16)
    w2_all[:H2, 1] = w2m[H1:].astype(np.float16)
    w2_all[:, 2] = w2p[:H1].astype(np.float16)
    for g in range(4):
        w2_all[32 * g:32 * g + H2, 3] = w2p[H1:].astype(np.float16)

    shared = {
        "w1a": padD(W1a),
        "w1b": padD(W1b),
        "w1c": padD(W1c),
        "w1m": padD(np.asarray(inputs["w1m"], dtype=np.float32)),
        "bias_all": bias_all,
        "w2_all": w2_all,
        "idn": np.eye(128, dtype=np.float16),
    }

    b2m = float(np.asarray(inputs["b2m"]).reshape(-1)[0])
    b2p = float(np.asarray(inputs["b2p"]).reshape(-1)[0])
    in_maps = []
    for c in range(NCORES):
        r0 = MC * c
        xt_c = np.ascontiguousarray(xtp[:, r0:r0 + W])
        mmul = np.ones((MC, K + 1), dtype=np.float32)
        madd = np.full((MC, K + 1), np.float32(b2p + 2.0 * b2m), dtype=np.float32)
        mmul[:, K] = 0.0
        madd[:, K] = 0.0
        if c == 0:
            for i in range(min(K, MC)):
                mmul[i, :K - i] = 0.0
                madd[i, :K - i] = np.float32(-1e9)
        mm_ma = np.zeros((128, 4 * (K + 1)), dtype=np.float32)
        for mb in range(2):
            mm_ma[:, (K + 1) * mb:(K + 1) * (mb + 1)] = \
                mmul[128 * mb:128 * (mb + 1)]
            mm_ma[:, (K + 1) * (2 + mb):(K + 1) * (3 + mb)] = \
                madd[128 * mb:128 * (mb + 1)]
        in_maps.append({"xt": xt_c, "mm_ma": mm_ma, **shared})
    return in_maps


def _get_nc(inputs):
    if "nc" not in _cache:
        _cache["nc"] = _build()
    return _cache["nc"]


def _run(inputs, trace=False):
    assert int(np.asarray(inputs["K"])) == K
    nc = _get_nc(inputs)
    in_maps = _prep_inputs(inputs)
    res = run_bass_kernel_spmd(nc, in_maps, list(range(NCORES)), trace=trace)
    out = np.concatenate([res.results[c]["out"] for c in range(NCORES)], axis=0)
    return out.astype(np.float32), res


def kernel(**inputs) -> np.ndarray:
    out, _ = _run(inputs, trace=False)
    return out


# revision 23
# speedup vs baseline: 1.5140x; 1.0187x over previous
"""CorefScore kernel for 8 Trainium2 NeuronCores.

Shards the mention axis M=2048 across 8 cores (256 owned mentions plus a
64-row halo of preceding mentions). The banded pairwise MLP runs in
"superrounds" of up to 4 rounds (2 deltas each): the DVE emits one batched
shifted-product tensor_tensor per d-tile per superround ([128, 8*256] fp16),
the PE contracts them with W1c in fp16 (fp32 PSUM); the Ya + shift(Yb) terms
are injected straight into PSUM via identity matmuls with strided moving
operands (no DVE merge for the 128-wide h half). The 22-wide h2 half of all
4 rounds is packed into one PSUM bank at partition offsets 0/32/64/96 via
col-tiled matmuls that execute concurrently on disjoint PE column strips.
Pair scores (w2p contraction) use diagonal 32x32 tiles, also packed 4 rounds
per PSUM bank, and are respread/transposed per superround so only a tiny
tail remains after the last round. Inputs load via a handful of large DMAs
split across the Sync and ScalarE HWDGE queues.
"""

import os
import sys

import numpy as np

for _p in ("/opt/trn_rl_repo", "/opt/pypackages"):
    if os.path.isdir(_p) and _p not in sys.path:
        sys.path.append(_p)

import concourse.bacc as bacc
import concourse.bass as bass
import concourse.mybir as mybir
import concourse.tile as tile
from concourse.ap import AP
from concourse.bass_utils import run_bass_kernel_spmd

F16 = mybir.dt.float16
F32 = mybir.dt.float32
AF = mybir.ActivationFunctionType

M, D, H, K = 2048, 900, 150, 50
NCORES = 8
MC = M // NCORES          # owned mentions per core
HB = 64                   # halo columns (>= K)
W = MC + HB               # X^T window width per core
DP = 1024                 # padded feature dim (8 tiles of 128)
NDT = DP // 128           # number of d tiles
H1, H2 = 128, H - 128     # h split
# superround sizes (rounds of 2 deltas each); sum = 25 rounds = 50 deltas;
# last kept smallest so the epilogue pair/respread chain is short
SRS = [3, 4, 4, 4, 4, 4, 2]

_cache = {}


def _ap3(t_ap, p_lo, p_n, off, dims, pstep=1):
    """3-D free-dim view of a tile AP: partitions [p_lo, p_lo+p_n*pstep) with
    partition step pstep, free offset `off` elements, free dims."""
    b = t_ap[p_lo:p_lo + 1, 0:1]
    pstride = b.ap[0][0]
    return AP(b.tensor, b.offset + off,
              [[pstride * pstep, p_n]] + [list(d) for d in dims])


def _build():
    nc = bacc.Bacc("TRN2", target_bir_lowering=False, debug=False)

    xt_d = nc.dram_tensor("xt", [DP, W], F16, kind="ExternalInput").ap()
    w1a_d = nc.dram_tensor("w1a", [DP, H], F16, kind="ExternalInput").ap()
    w1b_d = nc.dram_tensor("w1b", [DP, H], F16, kind="ExternalInput").ap()
    w1c_d = nc.dram_tensor("w1c", [DP, H], F16, kind="ExternalInput").ap()
    w1m_d = nc.dram_tensor("w1m", [DP, H], F16, kind="ExternalInput").ap()
    bias_d = nc.dram_tensor("bias_all", [128, 4], F32, kind="ExternalInput").ap()
    w2_d = nc.dram_tensor("w2_all", [128, 4], F16, kind="ExternalInput").ap()
    idn_d = nc.dram_tensor("idn", [128, 128], F16, kind="ExternalInput").ap()
    mm_d = nc.dram_tensor("mm_ma", [128, 4 * (K + 1)], F32,
                          kind="ExternalInput").ap()
    out_d = nc.dram_tensor("out", [MC, K + 1], F32, kind="ExternalOutput").ap()

    def dma3(eng, dst_tile, src_dram, cols, t0=0, t1=NDT):
        """One DMA loading d-tiles [t0, t1) of [DP, cols] dram into a
        [128, NDT*cols] tile at the matching free offset."""
        src = AP(src_dram.tensor, src_dram.offset + 128 * cols * t0,
                 [[cols, 128], [128 * cols, t1 - t0], [1, cols]])
        eng.dma_start(out=dst_tile[:, cols * t0:cols * t1], in_=src)

    with tile.TileContext(nc) as tc:
        with (
            tc.tile_pool(name="cp", bufs=1) as cp,
            tc.tile_pool(name="wp", bufs=2) as wp,
            tc.tile_pool(name="pp", bufs=2, space="PSUM") as pp,
        ):
            # ---- input loads: sync queue (xt split across both queues;
            # w1a first on the scalar queue so the Ya stream starts early) ----
            xt = cp.tile([128, NDT * W], F16, tag="xt")
            dma3(nc.sync, xt, xt_d, W, 0, 2)
            w1a = cp.tile([128, NDT * H], F16, tag="w1a")
            dma3(nc.scalar, w1a, w1a_d, H)
            dma3(nc.sync, xt, xt_d, W, 2, 4)
            dma3(nc.scalar, xt, xt_d, W, 4, 6)
            dma3(nc.scalar, xt, xt_d, W, 6, NDT)
            bias = cp.tile([128, 4], F32, tag="bias")
            nc.sync.dma_start(out=bias[:], in_=bias_d[:])
            idn = cp.tile([128, 128], F16, tag="idn")
            nc.sync.dma_start(out=idn[:], in_=idn_d[:])
            w2 = cp.tile([128, 4], F16, tag="w2")
            nc.sync.dma_start(out=w2[:], in_=w2_d[:])
            mm_ma = cp.tile([128, 4 * (K + 1)], F32, tag="mm_ma")
            nc.sync.dma_start(out=mm_ma[:], in_=mm_d[:])
            # ---- input loads: scalar (Act) HWDGE queue ----
            w1b = cp.tile([128, NDT * H], F16, tag="w1b")
            dma3(nc.scalar, w1b, w1b_d, H)
            w1m = cp.tile([128, NDT * H], F16, tag="w1m")
            dma3(nc.scalar, w1m, w1m_d, H)
            w1c = cp.tile([128, NDT * H], F16, tag="w1c")
            dma3(nc.scalar, w1c, w1c_d, H)

            def wsl(wt, t, ho, hn):  # stationary slice of a weight tile
                return wt[:, H * t + ho:H * t + ho + hn]

            # ---- preamble MLPs: Ya (owned, +b1p), Yb (window), mention ----
            psya1 = pp.tile([H1, MC], F32, tag="ah1")
            psya2 = pp.tile([H2, MC], F32, tag="ah1")
            psyb1 = pp.tile([H1, W], F32, tag="ah2")
            psyb2 = pp.tile([H2, W], F32, tag="ah2")
            psm1 = pp.tile([H1, W], F32, tag="psp")
            psm2 = pp.tile([H2, W], F32, tag="psp")
            for t in range(NDT):
                nc.tensor.matmul(psya1[:], wsl(w1a, t, 0, H1),
                                 xt[:, W * t + HB:W * (t + 1)],
                                 start=(t == 0), stop=(t == NDT - 1))
            for t in range(NDT):
                nc.tensor.matmul(psya2[:], wsl(w1a, t, H1, H2),
                                 xt[:, W * t + HB:W * (t + 1)],
                                 start=(t == 0), stop=(t == NDT - 1))
            for t in range(NDT):
                nc.tensor.matmul(psyb1[:], wsl(w1b, t, 0, H1),
                                 xt[:, W * t:W * (t + 1)],
                                 start=(t == 0), stop=(t == NDT - 1))
            for t in range(NDT):
                nc.tensor.matmul(psyb2[:], wsl(w1b, t, H1, H2),
                                 xt[:, W * t:W * (t + 1)],
                                 start=(t == 0), stop=(t == NDT - 1))
            for t in range(NDT):
                nc.tensor.matmul(psm1[:], wsl(w1m, t, 0, H1),
                                 xt[:, W * t:W * (t + 1)],
                                 start=(t == 0), stop=(t == NDT - 1))
            for t in range(NDT):
                nc.tensor.matmul(psm2[:], wsl(w1m, t, H1, H2),
                                 xt[:, W * t:W * (t + 1)],
                                 start=(t == 0), stop=(t == NDT - 1))

            ya1 = cp.tile([H1, MC], F16, tag="ya1")
            nc.scalar.activation(ya1[:], psya1[:], AF.Identity, bias=bias[:, 2:3])
            ya2 = cp.tile([H2, MC], F16, tag="ya2")
            nc.scalar.activation(ya2[:], psya2[:], AF.Identity,
                                 bias=bias[0:H2, 3:4])
            yb1 = cp.tile([H1, W], F16, tag="yb1")
            nc.scalar.copy(yb1[:], psyb1[:])
            yb2 = cp.tile([H2, W], F16, tag="yb2")
            nc.scalar.copy(yb2[:], psyb2[:])
            ma1 = cp.tile([H1, W], F16, tag="ma1")
            nc.scalar.activation(ma1[:], psm1[:], AF.Relu, bias=bias[:, 0:1])
            ma2 = cp.tile([H2, W], F16, tag="ma2")
            nc.scalar.activation(ma2[:], psm2[:], AF.Relu, bias=bias[0:H2, 1:2])

            # mention score row over the window
            psme = pp.tile([1, W], F32, tag="pre")
            nc.tensor.matmul(psme[:], w2[:, 0:1], ma1[:], start=True, stop=False)
            nc.tensor.matmul(psme[:], w2[0:H2, 1:2], ma2[:], start=False,
                             stop=True)
            # row 0 = mention scores; rows 1..31 stay zero so the e0-column
            # identity-inject matmul (32-wide contraction) picks row 0 only
            ment_row = cp.tile([32, W], F16, tag="ment_row")
            nc.vector.memset(ment_row[:], 0.0)
            nc.scalar.copy(ment_row[0:1, :], psme[:])
            # ment as per-partition columns for the owned 2x128 mention blocks
            ment_col = []
            for mb in range(2):
                pst = pp.tile([128, 1], F16, tag="pre")
                nc.tensor.transpose(pst[:],
                                    ment_row[0:1, HB + 128 * mb:HB + 128 * (mb + 1)],
                                    idn[0:1, 0:1])
                mcol = cp.tile([128, 1], F32, tag=f"mcol{mb}")
                nc.scalar.copy(mcol[:], pst[:])
                ment_col.append(mcol)

            scm = []
            for mb in range(2):
                s = cp.tile([128, K + 1], F32, tag=f"scm{mb}")
                nc.vector.memset(s[:], 0.0)
                scm.append(s)

            # ---- DVE: batched shifted products per superround ----
            # SR s covers rounds r0..r0+R-1; delta of column group j (0..2R-1)
            # is d0 - j with d0 = K - 2*r0; product col j*MC+m multiplies
            # X^T[., m] (owned) by X^T[., m - (d0 - j)].
            r0s = []
            acc = 0
            for R in SRS:
                r0s.append(acc)
                acc += R

            def emit_products(s):
                R = SRS[s]
                d0 = K - 2 * r0s[s]
                pts = []
                for t in range(NDT):
                    pt = wp.tile([128, 2 * R * MC], F16, tag=f"pt{t}",
                                 padded_shape=[128, 8 * MC])
                    nc.vector.tensor_tensor(
                        _ap3(pt[:], 0, 128, 0, [(MC, 2 * R), (1, MC)]),
                        _ap3(xt[:], 0, 128, W * t + HB, [(0, 2 * R), (1, MC)]),
                        _ap3(xt[:], 0, 128, W * t + HB - d0, [(1, 2 * R), (1, MC)]),
                        mybir.AluOpType.mult)
                    pts.append(pt)
                return pts

            def emit_c2(s):
                R = SRS[s]
                d0 = K - 2 * r0s[s]
                c2 = wp.tile([H2, 2 * R * MC], F16, tag="c2",
                             padded_shape=[128, 8 * MC])
                nc.vector.tensor_tensor(
                    _ap3(c2[:], 0, H2, 0, [(MC, 2 * R), (1, MC)]),
                    _ap3(ya2[:], 0, H2, 0, [(0, 2 * R), (1, MC)]),
                    _ap3(yb2[:], 0, H2, HB - d0, [(1, 2 * R), (1, MC)]),
                    mybir.AluOpType.add)
                return c2

            def emit_c1(s):
                R = SRS[s]
                d0 = K - 2 * r0s[s]
                c1 = wp.tile([H1, 2 * R * MC], F16, tag="c1",
                             padded_shape=[128, 8 * MC])
                nc.vector.tensor_tensor(
                    _ap3(c1[:], 0, H1, 0, [(MC, 2 * R), (1, MC)]),
                    _ap3(ya1[:], 0, H1, 0, [(0, 2 * R), (1, MC)]),
                    _ap3(yb1[:], 0, H1, HB - d0, [(1, 2 * R), (1, MC)]),
                    mybir.AluOpType.add)
                return c1

            pts_q = {0: emit_products(0)}
            c1_q = {0: emit_c1(0)}
            c2_q = {0: emit_c2(0)}
            pts_q[1] = emit_products(1)

            # ---- superround loop state ----
            a1_ring = []      # (sr, g) -> a1 tile, kept one SR back
            prev = None       # state of SR s-1 for deferred pair work

            pairK_pool = wp   # per-SR respread tiles

            def emit_pairs_for(state):
                """PE pair matmuls + evac + respread + transposes for SR s-1."""
                s, R, a1s, a2xs = state
                d0 = K - 2 * r0s[s]
                psp = pp.tile([97, MC * 2], F32, tag="psp")
                for g in range(R):
                    # ment_j first (start=True): a strided-moving matmul with
                    # start=False wedges the exec unit, so it opens the group
                    nc.tensor.matmul(
                        psp[32 * g:32 * g + 1, :], idn[0:32, 0:1],
                        _ap3(ment_row[:], 0, 32, HB - (d0 - 2 * g),
                             [(1, 2), (1, MC)]),
                        start=True, stop=False, tile_position=(0, 32 * g))
                    nc.tensor.matmul(psp[32 * g:32 * g + 1, :], w2[:, 2:3],
                                     a1s[g][:], start=False, stop=False,
                                     tile_position=(0, 32 * g))
                    nc.tensor.matmul(psp[32 * g:32 * g + 1, :],
                                     w2[32 * g:32 * g + H2, 3:4],
                                     a2xs[32 * g:32 * g + H2, :],
                                     start=False, stop=True,
                                     tile_position=(32 * g, 32 * g))
                pair_sb = wp.tile([97, MC * 2], F16, tag="pair_sb")
                nc.scalar.copy(pair_sb[:], psp[:])
                pairK = pairK_pool.tile([2 * R, MC], F16, tag="pairK",
                                        padded_shape=[128, MC])
                nc.sync.dma_start(
                    out=pairK[:],
                    in_=_ap3(pair_sb[:], 0, R, 0, [(MC, 2), (1, MC)], pstep=32))
                return pairK

            def emit_respread_tail(state, pairK):
                """Transposes + scM chunk evac for SR s-1 (after its DMA)."""
                s, R, a1s, a2xs = state
                k0 = 2 * r0s[s]
                cn = 2 * R
                for mb in range(2):
                    ptr = pp.tile([128, cn], F16, tag="pre",
                                  padded_shape=[128, 8])
                    nc.tensor.transpose(ptr[:], pairK[0:cn, 128 * mb:128 * (mb + 1)],
                                        idn[0:cn, 0:cn])
                    nc.scalar.activation(scm[mb][:, k0:k0 + cn], ptr[:],
                                         AF.Identity, bias=ment_col[mb][:])

            for s, R in enumerate(SRS):
                r0 = r0s[s]
                d0 = K - 2 * r0
                pts = pts_q.pop(s)
                c1 = c1_q.pop(s)
                c2 = c2_q.pop(s)

                # deferred pair matmuls for the previous superround
                pairK_prev = emit_pairs_for(prev) if prev is not None else None

                # per-round h1 streams (+ Ya / shifted-Yb injections)
                a1s = []
                for g in range(R):
                    ps1 = pp.tile([H1, 2 * MC], F32, tag="ah1")
                    for t in range(NDT):
                        nc.tensor.matmul(ps1[:], wsl(w1c, t, 0, H1),
                                         pts[t][:, 2 * g * MC:(2 * g + 2) * MC],
                                         start=(t == 0), stop=False)
                    nc.tensor.matmul(
                        ps1[:], idn[:], c1[:, 2 * g * MC:(2 * g + 2) * MC],
                        start=False, stop=True)
                    a1 = wp.tile([H1, 2 * MC], F16, tag="a1", bufs=8)
                    nc.scalar.activation(a1[:], ps1[:], AF.Relu)
                    a1s.append(a1)

                # packed h2: col-tiled matmuls, 4 rounds -> one PSUM bank
                hp = 32 * (R - 1) + H2
                ps2 = pp.tile([hp, 2 * MC], F32, tag="ah2")
                for t in range(NDT):
                    for g in range(R):
                        nc.tensor.matmul(ps2[32 * g:32 * g + H2, :],
                                         wsl(w1c, t, H1, H2),
                                         pts[t][:, 2 * g * MC:(2 * g + 2) * MC],
                                         start=(t == 0), stop=False,
                                         tile_position=(0, 32 * g))
                for g in range(R):
                    nc.tensor.matmul(ps2[32 * g:32 * g + H2, :],
                                     idn[0:H2, 0:H2],
                                     c2[0:H2, 2 * g * MC:(2 * g + 2) * MC],
                                     start=False, stop=True,
                                     tile_position=(0, 32 * g))
                a2x = wp.tile([hp, 2 * MC], F16, tag="a2x",
                              padded_shape=[128, 2 * MC])
                nc.scalar.activation(a2x[:], ps2[:], AF.Relu)

                # respread tail for SR s-1 (its DMA has landed by now)
                if prev is not None:
                    emit_respread_tail(prev, pairK_prev)

                # prefetch DVE work for s+2 / c2 for s+1
                if s + 1 < len(SRS):
                    c1_q[s + 1] = emit_c1(s + 1)
                    c2_q[s + 1] = emit_c2(s + 1)
                if s + 2 < len(SRS):
                    pts_q[s + 2] = emit_products(s + 2)

                prev = (s, R, a1s, a2x[:])

            # ---- epilogue: pairs of the last superround + final masking ----
            pairK_last = emit_pairs_for(prev)
            emit_respread_tail(prev, pairK_last)
            for mb in range(2):
                nc.vector.tensor_mul(scm[mb][:], scm[mb][:],
                                     mm_ma[:, (K + 1) * mb:(K + 1) * (mb + 1)])
                nc.vector.tensor_add(
                    scm[mb][:], scm[mb][:],
                    mm_ma[:, (K + 1) * (2 + mb):(K + 1) * (3 + mb)])
                nc.sync.dma_start(out=out_d[128 * mb:128 * (mb + 1), :],
                                  in_=scm[mb][:])

    nc.compile()
    return nc


def _prep_inputs(inputs):
    X = np.ascontiguousarray(inputs["mention_reprs"], dtype=np.float32)
    assert X.shape == (M, D)
    w1p = np.asarray(inputs["w1p"], dtype=np.float32)
    W1a, W1b, W1c = w1p[:D], w1p[D:2 * D], w1p[2 * D:]

    def padD(w):  # [D, H] -> [DP, H] fp16
        out = np.zeros((DP, H), dtype=np.float16)
        out[:D] = w.astype(np.float16)
        return out

    xtp = np.zeros((DP, M + HB), dtype=np.float16)
    xtp[:D, HB:] = X.T.astype(np.float16)

    b1m = np.asarray(inputs["b1m"], dtype=np.float32)
    b1p = np.asarray(inputs["b1p"], dtype=np.float32)
    bias_all = np.zeros((128, 4), dtype=np.float32)
    bias_all[:, 0] = b1m[:H1]
    bias_all[:H2, 1] = b1m[H1:]
    bias_all[:, 2] = b1p[:H1]
    bias_all[:H2, 3] = b1p[H1:]

    w2m = np.asarray(inputs["w2m"], dtype=np.float32)
    w2p = np.asarray(inputs["w2p"], dtype=np.float32)
    w2_all = np.zeros((128, 4), dtype=np.float16)
    w2_all[:, 0] = w2m[:H1].astype(np.float16)
    w2_all[:H2, 1] = w2m[H1:].astype(np.float16)
    w2_all[:, 2] = w2p[:H1].astype(np.float16)
    for g in range(4):
        w2_all[32 * g:32 * g + H2, 3] = w2p[H1:].astype(np.float16)

    shared = {
        "w1a": padD(W1a),
        "w1b": padD(W1b),
        "w1c": padD(W1c),
        "w1m": padD(np.asarray(inputs["w1m"], dtype=np.float32)),
        "bias_all": bias_all,
        "w2_all": w2_all,
        "idn": np.eye(128, dtype=np.float16),
    }

    b2m = float(np.asarray(inputs["b2m"]).reshape(-1)[0])
    b2p = float(np.asarray(inputs["b2p"]).reshape(-1)[0])
    in_maps = []
    for c in range(NCORES):
        r0 = MC * c
        xt_c = np.ascontiguousarray(xtp[:, r0:r0 + W])
        mmul = np.ones((MC, K + 1), dtype=np.float32)
        madd = np.full((MC, K + 1), np.float32(b2p + 2.0 * b2m), dtype=np.float32)
        mmul[:, K] = 0.0
        madd[:, K] = 0.0
        if c == 0:
            for i in range(min(K, MC)):
                mmul[i, :K - i] = 0.0
                madd[i, :K - i] = np.float32(-1e9)
        mm_ma = np.zeros((128, 4 * (K + 1)), dtype=np.float32)
        for mb in range(2):
            mm_ma[:, (K + 1) * mb:(K + 1) * (mb + 1)] = \
                mmul[128 * mb:128 * (mb + 1)]
            mm_ma[:, (K + 1) * (2 + mb):(K + 1) * (3 + mb)] = \
                madd[128 * mb:128 * (mb + 1)]
        in_maps.append({"xt": xt_c, "mm_ma": mm_ma, **shared})
    return in_maps


def _get_nc(inputs):
    if "nc" not in _cache:
        _cache["nc"] = _build()
    return _cache["nc"]


def _run(inputs, trace=False):
    assert int(np.asarray(inputs["K"])) == K
    nc = _get_nc(inputs)
    in_maps = _prep_inputs(inputs)
    res = run_bass_kernel_spmd(nc, in_maps, list(range(NCORES)), trace=trace)
    out = np.concatenate([res.results[c]["out"] for c in range(NCORES)], axis=0)
    return out.astype(np.float32), res


def kernel(**inputs) -> np.ndarray:
    out, _ = _run(inputs, trace=False)
    return out
